# revision 40
# baseline (speedup 1.0000x reference)
"""Adaptive-softmax NLL loss on 8 Trainium2 NeuronCores.

Algorithm (cluster-sparse): per token only its own cluster's log-softmax
matters, so
    nll[t] = -( cl[t, c(t)] - LSE(cl[t,:]) + logit[t, y_t] - ln S[t] )
with  S[t] = sum_{j in cluster(y_t)} exp(x_t . W[:,j] + b_j).

Sharding: tokens are cluster-sorted into 128-row blocks; each cluster's
vocab range is split evenly across the 8 cores (tensor parallel over
vocab).  Every core computes partial S for all tokens over its vocab
slice (fp8 DoubleRow matmul -> ScalarE exp with free-axis accumulate),
the partials are combined with a single small AllReduce, and each core
finishes the per-token epilogue locally.  The target logit is computed
from the host-gathered columns W[:, y] as an elementwise bf16 dot on
VectorE, as is the 3-column cluster head.  fp8 inputs are pre-scaled by
powers of two on the host; the exp's built-in scale multiplier unwinds
the scaling for free.  The odd-sized tail group of each big-cluster
block is exp-summed on VectorE via a Schraudolph bit-trick to keep
ScalarE below the TensorE floor, and dummy AllReduces warm the
collective path so the real one runs at its warm latency.
"""

import numpy as np
import ml_dtypes
from contextlib import ExitStack

import concourse.bass as bass
import concourse.mybir as mybir
from concourse.bass_utils import run_bass_kernel_spmd

F32 = mybir.dt.float32
I32 = mybir.dt.int32
BF16 = mybir.dt.bfloat16
FP8 = mybir.dt.float8e4
AF = mybir.ActivationFunctionType
ALU = mybir.AluOpType
DR = mybir.MatmulPerfMode.DoubleRow
DRSW = mybir.MatmulPerfMode.DoubleRowSwInterleave
USE_SWI = False

N_CORES = 8
PART = 128
CUTOFFS = [0, 2000, 10000, 50000]
HID = 512

GROUP_COLS = 1024   # retained for the small-scale sim configs
TCAPS = [1024, 1024, 1024, 1024]   # psum tensor widths (2 banks each)
MM_F = 512          # max matmul free size (one psum bank)
SCALE_W = 2048.0    # fp8 pre-scale for weights (power of 2)
SCALE_X = 32.0      # fp8 pre-scale for activations (power of 2)

DISABLE = set()     # bisection hooks


# ---------------------------------------------------------------------------
# planning


class Plan:
    """Static schedule shared by the host sharding code and the builder."""

    def __init__(self, blocks_per_cluster, widths, has_bias, group_cols=GROUP_COLS,
                 hid=HID, mm_f=MM_F):
        assert hid % 256 == 0
        self.hg = hid // PART          # 128-row h-groups (4)
        self.ndr = hid // 256          # DoubleRow matmuls per unit (2)
        self.hid = hid
        self.has_bias = has_bias
        self.group_cols = group_cols
        self.mm_f = mm_f
        self.widths = widths                      # per-core cols per cluster
        self.bpc = blocks_per_cluster             # blocks per cluster
        self.nb = sum(blocks_per_cluster)
        self.ncl = len(widths)
        self.act_scale = 1.0 / (SCALE_W * SCALE_X)

        # per-core w column layout: [c0 | c1 | ... ] (cluster head is
        # computed on VectorE from bf16 inputs instead)
        self.w_off = []
        off = 0
        for wd in widths:
            self.w_off.append(off)
            off += wd
        self.wcols = off

        # head-split: part0 = first cluster only; part1 = all clusters
        # except the last; part2 = the big last cluster
        self.wsplit0 = sum(widths[:-1])
        self.tsplit0 = PART * sum(blocks_per_cluster[:-1])
        self.wsplit = self.w_off[-1]
        self.tsplit = PART * sum(blocks_per_cluster[:-1])

        # blocks: cluster index per block
        self.block_cluster = []
        for ci, nblk in enumerate(blocks_per_cluster):
            self.block_cluster += [ci] * nblk

        # groups: the unit of PSUM rotation.  Asymmetric psum tensors,
        # assigned round-robin (LRU); each group is one ACT exp+accum.
        if group_cols == GROUP_COLS:
            self.tcaps = list(TCAPS)
        else:                      # small-scale sim: 4 tensors of group_cols
            self.tcaps = [group_cols] * 4
        self.groups = []   # dicts: b, gi, tidx, prev_g, units[(po,wo,F)], span
        lru = list(range(len(self.tcaps)))
        last_on = [None] * len(self.tcaps)
        for b, ci in enumerate(self.block_cluster):
            V = widths[ci]
            wo0 = self.w_off[ci]
            col = 0
            gi = 0
            while col < V:
                t = lru.pop(0)
                lru.append(t)
                gsz = min(self.tcaps[t], V - col)
                units = []
                po = 0
                rem = gsz
                while rem > 0:
                    f = min(self.mm_f, rem)
                    units.append((po, wo0 + col + po, f))
                    po += f
                    rem -= f
                g = len(self.groups)
                self.groups.append(dict(b=b, gi=gi, tidx=t, prev_g=last_on[t],
                                        units=units, span=gsz))
                last_on[t] = g
                col += gsz
                gi += 1
        self.ngroups = len(self.groups)

        # fuse ACT over pairs of full-cap groups in adjacent psum quarters
        # (the psum is one contiguous tensor; consecutive tidx => contiguous
        # columns).  The odd-sized tail group of each last-cluster block is
        # emitted unfused and offloaded to VectorE (Schraudolph exp).
        self.act_instrs = []    # dicts: b, span_off, span, slot, last_g, eng, ord
        self.act_of_group = [None] * self.ngroups
        caps = self.tcaps
        g = 0
        while g < self.ngroups:
            grp = self.groups[g]
            b = grp["b"]
            fuse = False
            if g + 1 < self.ngroups:
                nxt = self.groups[g + 1]
                if (nxt["b"] == b and nxt["tidx"] == grp["tidx"] + 1
                        and grp["span"] == caps[grp["tidx"]]
                        and nxt["span"] == caps[nxt["tidx"]]):
                    fuse = True
            off = sum(caps[:grp["tidx"]])
            idx = len(self.act_instrs)
            slot = len([a for a in self.act_instrs if a["b"] == b])
            if fuse:
                span = grp["span"] + self.groups[g + 1]["span"]
                self.act_of_group[g] = idx
                self.act_of_group[g + 1] = idx
                self.act_instrs.append(dict(b=b, span_off=off, span=span,
                                            slot=slot, last_g=g + 1, eng="act"))
                g += 2
            else:
                self.act_of_group[g] = idx
                self.act_instrs.append(dict(b=b, span_off=off, span=grp["span"],
                                            slot=slot, last_g=g, eng="act"))
                g += 1
        # offload: the final (always unfused) instr of each last-cluster block
        last_of_block = {}
        for a in self.act_instrs:
            last_of_block[a["b"]] = a
        for b, a in last_of_block.items():
            if self.block_cluster[b] == self.ncl - 1 and a["span"] < max(caps):
                a["eng"] = "dve"
        # per-engine ordinals
        na = nd = 0
        for a in self.act_instrs:
            if a["eng"] == "act":
                a["ord"] = na
                na += 1
            else:
                a["ord"] = nd
                nd += 1
        self.n_act_eng = na
        self.n_dvx = nd
        # per-block list of dve-offloaded instrs
        self.dvx_of_block = {}
        for a in self.act_instrs:
            if a["eng"] == "dve":
                self.dvx_of_block.setdefault(a["b"], []).append(a)
        self.n_act = len(self.act_instrs)
        self.max_gpb = max(a["slot"] for a in self.act_instrs) + 1
        # Schraudolph constants for the DVE exp offload
        self.dve_a = self.act_scale * (2.0 ** 23) / float(np.log(2.0))
        self.dve_b = 127.0 * 2 ** 23 - 486411.0

        # first group needing part1 (middle clusters) / part2 (last cluster)
        self.first_p1_group = None
        self.first_p2_group = None
        for g, grp in enumerate(self.groups):
            ci = self.block_cluster[grp["b"]]
            if ci not in (0, self.ncl - 1) and self.first_p1_group is None:
                self.first_p1_group = g
            if ci == self.ncl - 1 and self.first_p2_group is None:
                self.first_p2_group = g
                break


def build_graph(plan: Plan):
    nc = bass.Bass()
    HG, NB, G = plan.hg, plan.nb, plan.ngroups
    NTOK = NB * PART
    W = plan.wcols
    WS, TS = plan.wsplit, plan.tsplit

    if USE_SWI:
        x8_ext = nc.declare_dram_parameter("x8", [PART, NB, plan.ndr, 2 * PART],
                                           FP8, isOutput=False)
    else:
        x8_ext = nc.declare_dram_parameter("x8", [PART, HG, NTOK], FP8,
                                           isOutput=False)
    w8_ext = nc.declare_dram_parameter("w8", [PART, HG, W], FP8, isOutput=False)
    xe_ext = nc.declare_dram_parameter("xe", [NTOK, plan.hid], BF16, isOutput=False)
    wt_ext = nc.declare_dram_parameter("wt", [NTOK, plan.hid], BF16, isOutput=False)
    cwb_ext = nc.declare_dram_parameter("cwb", [PART, 3 * plan.hid], BF16,
                                        isOutput=False)
    oh_ext = nc.declare_dram_parameter("oh", [PART, NB, 3], F32, isOutput=False)
    bt_ext = nc.declare_dram_parameter("bt", [PART, NB], F32, isOutput=False)
    if plan.has_bias:
        brow_ext = nc.declare_dram_parameter("brow", [1, W], BF16, isOutput=False)
        clb_ext = nc.declare_dram_parameter("clb", [PART, NB, 3], F32,
                                            isOutput=False)
    out_ext = nc.declare_dram_parameter("out", [PART, NB], F32, isOutput=True)

    ar_in = nc.dram_tensor("ar_in", [PART, NB], F32)
    ar_out = nc.dram_tensor("ar_out", [PART, NB], F32, addr_space="Shared")
    dm_in = nc.dram_tensor("dm_in", [PART], F32)
    dm_out = nc.dram_tensor("dm_out", [PART], F32, addr_space="Shared")

    n_p0 = 1 + (1 if plan.tsplit0 > 0 else 0) + (1 if plan.has_bias else 0)
    n_p1 = ((1 if plan.wsplit > plan.wsplit0 else 0)
            + (1 if plan.tsplit > plan.tsplit0 else 0))
    n_misc = 2                               # oh, bt

    with ExitStack() as ctx:
        w8_sb = ctx.enter_context(nc.sbuf_tensor([PART, HG, W], FP8))
        if USE_SWI:
            x8_sb = ctx.enter_context(
                nc.sbuf_tensor([PART, NB * plan.ndr * 2 * PART], FP8))
        else:
            x8_sb = ctx.enter_context(nc.sbuf_tensor([PART, HG, NTOK], FP8))
        xe_sb = ctx.enter_context(nc.sbuf_tensor([PART, 2 * plan.hid], BF16))
        wt_sb = ctx.enter_context(nc.sbuf_tensor([PART, 2 * plan.hid], BF16))
        sacc_sb = ctx.enter_context(nc.sbuf_tensor([PART, NB, plan.max_gpb], F32))
        cl_sb = ctx.enter_context(nc.sbuf_tensor([PART, NB, 3], F32))
        ecl_sb = ctx.enter_context(nc.sbuf_tensor([PART, NB, 3], F32))
        oh_sb = ctx.enter_context(nc.sbuf_tensor([PART, NB, 3], F32))
        tmp3_sb = ctx.enter_context(nc.sbuf_tensor([PART, NB, 3], F32))
        prod_sb = ctx.enter_context(nc.sbuf_tensor([PART, 8 * plan.hid], F32))
        cwb_sb = ctx.enter_context(nc.sbuf_tensor([PART, 3 * plan.hid], BF16))
        t_sb = ctx.enter_context(nc.sbuf_tensor([PART, NB], F32))
        bt_sb = ctx.enter_context(nc.sbuf_tensor([PART, NB], F32))
        s_sb = ctx.enter_context(nc.sbuf_tensor([PART, NB], F32))
        st_sb = ctx.enter_context(nc.sbuf_tensor([PART, NB], F32))
        lns_sb = ctx.enter_context(nc.sbuf_tensor([PART, NB], F32))
        se3_sb = ctx.enter_context(nc.sbuf_tensor([PART, NB], F32))
        lse3_sb = ctx.enter_context(nc.sbuf_tensor([PART, NB], F32))
        clsel_sb = ctx.enter_context(nc.sbuf_tensor([PART, NB], F32))
        fin_sb = ctx.enter_context(nc.sbuf_tensor([PART, NB], F32))
        ones_sb = ctx.enter_context(nc.sbuf_tensor([1, PART], BF16))
        brow_sb = ctx.enter_context(nc.sbuf_tensor([1, W], BF16))
        ps = ctx.enter_context(nc.psum_tensor("ps",
                                              [PART, sum(plan.tcaps)], F32))
        pbase = [sum(plan.tcaps[:i]) for i in range(len(plan.tcaps))]
        dma_w0 = ctx.enter_context(nc.semaphore("dma_w0"))
        dma_w1 = ctx.enter_context(nc.semaphore("dma_w1"))
        dma_w2 = ctx.enter_context(nc.semaphore("dma_w2"))
        dma_misc = ctx.enter_context(nc.semaphore("dma_misc"))
        dma_ep0 = ctx.enter_context(nc.semaphore("dma_ep0"))
        dma_ep1 = ctx.enter_context(nc.semaphore("dma_ep1"))
        dma_out = ctx.enter_context(nc.semaphore("dma_out"))
        mm_sem = ctx.enter_context(nc.semaphore("mm_sem"))
        act_sem = ctx.enter_context(nc.semaphore("act_sem"))
        dma_cwb = ctx.enter_context(nc.semaphore("dma_cwb"))
        tdot_sem = ctx.enter_context(nc.semaphore("tdot_sem"))
        veini_sem = ctx.enter_context(nc.semaphore("veini_sem"))
        ve_sem = ctx.enter_context(nc.semaphore("ve_sem"))
        ve2_sem = ctx.enter_context(nc.semaphore("ve2_sem"))
        cc_sem = ctx.enter_context(nc.semaphore("cc_sem"))
        fin_sem = ctx.enter_context(nc.semaphore("fin_sem"))
        outv_sem = ctx.enter_context(nc.semaphore("outv_sem"))
        vchain_sem = ctx.enter_context(nc.semaphore("vchain_sem"))
        gp_sem = ctx.enter_context(nc.semaphore("gp_sem"))
        dvx_sem = ctx.enter_context(nc.semaphore("dvx_sem"))
        dvxp_sem = ctx.enter_context(nc.semaphore("dvxp_sem"))
        block = ctx.enter_context(nc.Block())

        WS0, TS0 = plan.wsplit0, plan.tsplit0

        @block.sync
        def _(sync):
            # part 0: just the first cluster's slice, to start PE asap
            sync.dma_start(out=w8_sb[:, :, 0:WS0],
                           in_=w8_ext[:, :, 0:WS0]).then_inc(dma_w0, 16)
            # part 1: remaining small clusters (empty when no middle part)
            if WS > WS0:
                sync.dma_start(out=w8_sb[:, :, WS0:WS],
                               in_=w8_ext[:, :, WS0:WS]).then_inc(dma_w1, 16)

            if plan.has_bias:
                sync.dma_start(out=brow_sb[:], in_=brow_ext[:]).then_inc(dma_w0, 16)
            sync.dma_start(out=cwb_sb[:], in_=cwb_ext[:]).then_inc(dma_cwb, 16)
            if plan.has_bias:
                sync.dma_start(out=tmp3_sb[:], in_=clb_ext[:]).then_inc(dma_cwb, 16)
            # part 2: the big cluster
            sync.dma_start(out=w8_sb[:, :, WS:W],
                           in_=w8_ext[:, :, WS:W]).then_inc(dma_w2, 16)

            # misc for the epilogue
            sync.dma_start(out=oh_sb[:], in_=oh_ext[:]).then_inc(dma_misc, 16)
            sync.dma_start(out=bt_sb[:], in_=bt_ext[:]).then_inc(dma_misc, 16)
            # epilogue tiles, double-buffered, paced by the t-dot consumer
            for e in range(NB):
                if e >= 2:
                    sync.wait_ge(tdot_sem, 4 * (e - 1))
                sem_e = dma_ep0 if e % 2 == 0 else dma_ep1
                toff = (e % 2) * plan.hid
                sync.dma_start(out=xe_sb[:, toff:toff + plan.hid],
                               in_=xe_ext[e * PART:(e + 1) * PART, :]
                               ).then_inc(sem_e, 16)
                sync.dma_start(out=wt_sb[:, toff:toff + plan.hid],
                               in_=wt_ext[e * PART:(e + 1) * PART, :]
                               ).then_inc(sem_e, 16)
            # S partials out, AllReduce result back, final output
            sync.wait_ge(ve_sem, 1)
            sync.dma_start(out=ar_in[:], in_=s_sb[:]).then_inc(dma_out, 16)
            sync.wait_ge(cc_sem, 3)
            sync.dma_start(out=st_sb[:], in_=ar_out[:]).then_inc(dma_out, 16)
            sync.wait_ge(outv_sem, 1)
            sync.dma_start(out=out_ext[:], in_=fin_sb[:]).then_inc(dma_out, 16)

        @block.gpsimd
        def _(gpsimd):
            # tiny dummy collective issued immediately: pays the cold-start
            # and entry-barrier cost concurrently with the main compute, so
            # the real AllReduce at the end runs on a warm path
            gpsimd.dma_start(out=dm_in[:],
                             in_=bt_ext[:].rearrange("p e -> (p e)")[0:PART]
                             ).then_inc(gp_sem, 16)
            gpsimd.wait_ge(gp_sem, 16)
            gpsimd.collective_compute(
                "AllReduce",
                ALU.add,
                ins=[dm_in[:]],
                outs=[dm_out[:]],
                replica_groups=[list(range(N_CORES))],
            ).then_inc(cc_sem, 1)
            gpsimd.wait_ge(mm_sem, (G * 11) // 20)
            gpsimd.collective_compute(
                "AllReduce",
                ALU.add,
                ins=[dm_in[:]],
                outs=[dm_out[:]],
                replica_groups=[list(range(N_CORES))],
            ).then_inc(cc_sem, 1)
            gpsimd.wait_ge(dma_out, 16)
            gpsimd.collective_compute(
                "AllReduce",
                ALU.add,
                ins=[ar_in[:]],
                outs=[ar_out[:]],
                replica_groups=[list(range(N_CORES))],
            ).then_inc(cc_sem, 1)

        @block.tensor
        def _(tensor):
            tensor.wait_ge(dma_w0, 16 * n_p0)
            if plan.has_bias:
                tensor.wait_ge(veini_sem, 2)  # ones row ready
            for g, grp in enumerate(plan.groups):
                pb0 = pbase[grp["tidx"]]
                if g == plan.first_p1_group and n_p1 > 0:
                    tensor.wait_ge(dma_w1, 16 * n_p1)
                if g == plan.first_p2_group:
                    tensor.wait_ge(dma_w2, 32)
                if grp["prev_g"] is not None:
                    pa = plan.act_instrs[plan.act_of_group[grp["prev_g"]]]
                    tensor.wait_ge(act_sem if pa["eng"] == "act" else dvx_sem,
                                   pa["ord"] + 1)
                b = grp["b"]
                nunits = len(grp["units"])
                # j-outer: consecutive matmuls share the same stationary
                # operand, letting the LDWEIGHTS prefetch/dedup logic help
                for j in range(plan.ndr):
                    if USE_SWI:
                        xoff = (b * plan.ndr + j) * 2 * PART
                        lhsT = x8_sb[:, xoff:xoff + 2 * PART]
                    else:
                        lhsT = x8_sb[:, 2 * j:2 * j + 2,
                                     b * PART:(b + 1) * PART]
                    for ui, (po, wo, f) in enumerate(grp["units"]):
                        mm = tensor.matmul(
                            ps[:, pb0 + po:pb0 + po + f],
                            lhsT=lhsT,
                            rhs=w8_sb[:, 2 * j:2 * j + 2, wo:wo + f],
                            start=(j == 0),
                            stop=(j == plan.ndr - 1 and not plan.has_bias),
                            skip_group_check=True,
                            perf_mode=DRSW if USE_SWI else DR)
                        if (j == plan.ndr - 1 and not plan.has_bias
                                and ui == nunits - 1):
                            mm.then_inc(mm_sem, 1)
                    if plan.has_bias:
                        mm = tensor.matmul(
                            ps[:, pb0 + po:pb0 + po + f],
                            lhsT=ones_sb[:],
                            rhs=brow_sb[0:1, wo:wo + f],
                            start=False, stop=True)
                        if ui == nunits - 1:
                            mm.then_inc(mm_sem, 1)

        @block.scalar
        def _(scalar):
            # x8 loads ride the ACT engine's parallel HWDGE ring
            def x8_dma(sem, tok_lo, tok_hi):
                if USE_SWI:
                    blo, bhi = tok_lo // PART, tok_hi // PART
                    clo, chi = blo * plan.ndr * 2 * PART, bhi * plan.ndr * 2 * PART
                    scalar.dma_start(
                        out=x8_sb[:, clo:chi],
                        in_=x8_ext[:, blo:bhi, :, :]).then_inc(sem, 16)
                else:
                    scalar.dma_start(out=x8_sb[:, :, tok_lo:tok_hi],
                                     in_=x8_ext[:, :, tok_lo:tok_hi]).then_inc(sem, 16)

            if TS0 > 0:
                x8_dma(dma_w0, 0, TS0)
            if TS > TS0:
                x8_dma(dma_w1, TS0, TS)
            x8_dma(dma_w2, TS, NTOK)
            scalar.wait_ge(veini_sem, 1)
            for a in plan.act_instrs:
                if a["eng"] != "act":
                    continue
                scalar.wait_ge(mm_sem, a["last_g"] + 1)
                o, sp = a["span_off"], a["span"]
                scalar.activation(
                    ps[:, o:o + sp],
                    ps[:, o:o + sp],
                    AF.Exp,
                    scale=plan.act_scale,
                    accum_out=sacc_sb[:, a["b"], a["slot"]:a["slot"] + 1],
                ).then_inc(act_sem, 1)
            # epilogue
            if plan.has_bias:
                scalar.wait_ge(ve2_sem, 2)
            else:
                scalar.wait_ge(tdot_sem, 4 * NB)
            scalar.activation(ecl_sb[:], cl_sb[:], AF.Exp).then_inc(fin_sem, 1)
            scalar.wait_ge(ve2_sem, 3 if plan.has_bias else 1)
            scalar.activation(lse3_sb[:], se3_sb[:], AF.Ln).then_inc(fin_sem, 1)
            scalar.wait_ge(dma_out, 32)
            scalar.activation(lns_sb[:], st_sb[:], AF.Ln).then_inc(fin_sem, 1)

        @block.vector
        def _(vector):
            vector.memset(sacc_sb[:], 0.0).then_inc(veini_sem, 1)
            if plan.has_bias:
                vector.memset(ones_sb[:], 1.0).then_inc(veini_sem, 1)
            vector.wait_ge(dma_cwb, 32 if plan.has_bias else 16)
            H = plan.hid
            for b in range(NB):
                # Schraudolph exp+sum for the previous block's offloaded tail
                # group goes first so its psum quarter frees as soon as the
                # matmuls finish
                for a in plan.dvx_of_block.get(b - 1, []):
                    o, sp = a["span_off"], a["span"]
                    vector.wait_ge(mm_sem, a["last_g"] + 1)
                    vector.tensor_scalar(
                        out=ps[:, o:o + sp].bitcast(I32),
                        in0=ps[:, o:o + sp],
                        scalar1=plan.dve_a,
                        scalar2=plan.dve_b,
                        op0=ALU.mult,
                        op1=ALU.add).then_inc(dvxp_sem, 1)
                    vector.wait_ge(dvxp_sem, a["ord"] + 1)
                    vector.reduce_sum(
                        sacc_sb[:, a["b"], a["slot"]:a["slot"] + 1],
                        ps[:, o:o + sp],
                        axis=mybir.AxisListType.X).then_inc(dvx_sem, 1)
                vector.wait_ge(dma_ep0 if b % 2 == 0 else dma_ep1,
                               32 * (b // 2 + 1))
                toff = (b % 2) * H
                # target-logit dot + 3 cluster-head dots, each with its own
                # scratch slot (WAW across tiles is ordered transitively via
                # the DMA pacing)
                po = (b % 2) * 4 * H
                vector.scalar_tensor_tensor(
                    out=prod_sb[:, po:po + H],
                    in0=xe_sb[:, toff:toff + H],
                    scalar=1.0,
                    in1=wt_sb[:, toff:toff + H],
                    op0=ALU.mult,
                    op1=ALU.mult,
                    accum_out=t_sb[:, b:b + 1],
                ).then_inc(tdot_sem, 1)
                for i in range(3):
                    vector.scalar_tensor_tensor(
                        out=prod_sb[:, po + (i + 1) * H:po + (i + 2) * H],
                        in0=xe_sb[:, toff:toff + H],
                        scalar=1.0,
                        in1=cwb_sb[:, i * H:(i + 1) * H],
                        op0=ALU.mult,
                        op1=ALU.mult,
                        accum_out=cl_sb[:, b, i:i + 1],
                    ).then_inc(tdot_sem, 1)
            for a in plan.dvx_of_block.get(NB - 1, []):
                o, sp = a["span_off"], a["span"]
                vector.wait_ge(mm_sem, a["last_g"] + 1)
                vector.tensor_scalar(
                    out=ps[:, o:o + sp].bitcast(I32),
                    in0=ps[:, o:o + sp],
                    scalar1=plan.dve_a,
                    scalar2=plan.dve_b,
                    op0=ALU.mult,
                    op1=ALU.add).then_inc(dvxp_sem, 1)
                vector.wait_ge(dvxp_sem, a["ord"] + 1)
                vector.reduce_sum(
                    sacc_sb[:, a["b"], a["slot"]:a["slot"] + 1],
                    ps[:, o:o + sp],
                    axis=mybir.AxisListType.X).then_inc(dvx_sem, 1)
            # ---- tail (serialized through vchain_sem for the race detector)
            vc = 0
            if plan.has_bias:
                # cl += cluster_b (clb staged in tmp3_sb)
                vector.wait_ge(tdot_sem, 4 * NB)
                vector.wait_ge(dma_cwb, 32)
                vector.tensor_tensor(cl_sb[:], cl_sb[:], tmp3_sb[:],
                                     ALU.add).then_inc(ve2_sem, 2)
            vector.wait_ge(act_sem, plan.n_act_eng)
            vector.wait_ge(dvx_sem, plan.n_dvx)
            vector.tensor_reduce(s_sb[:], sacc_sb[:], mybir.AxisListType.X,
                                 ALU.add).then_inc(ve_sem, 1)
            # cluster-head select (overlaps the AllReduce)
            vector.wait_ge(dma_misc, 16 * n_misc)
            if plan.has_bias:
                vector.wait_ge(ve2_sem, 2)
            else:
                vector.wait_ge(tdot_sem, 4 * NB)
            vector.tensor_tensor(tmp3_sb[:], cl_sb[:], oh_sb[:],
                                 ALU.mult).then_inc(vchain_sem, 1)
            vc += 1
            vector.wait_ge(vchain_sem, vc)
            vector.tensor_reduce(clsel_sb[:], tmp3_sb[:], mybir.AxisListType.X,
                                 ALU.add).then_inc(vchain_sem, 1)
            vc += 1
            vector.wait_ge(fin_sem, 1)
            vector.tensor_reduce(se3_sb[:], ecl_sb[:], mybir.AxisListType.X,
                                 ALU.add).then_inc(ve2_sem, 1)
            # pre-AR: w = lse3 - clsel - t - bt  (staged in lse3_sb)
            vector.wait_ge(fin_sem, 2)
            vector.scalar_tensor_tensor(out=lse3_sb[:], in0=lse3_sb[:], scalar=1.0,
                                        in1=clsel_sb[:], op0=ALU.mult,
                                        op1=ALU.subtract).then_inc(vchain_sem, 1)
            vc += 1
            vector.wait_ge(vchain_sem, vc)
            vector.scalar_tensor_tensor(out=lse3_sb[:], in0=lse3_sb[:], scalar=1.0,
                                        in1=t_sb[:], op0=ALU.mult,
                                        op1=ALU.subtract).then_inc(vchain_sem, 1)
            vc += 1
            vector.wait_ge(vchain_sem, vc)
            vector.scalar_tensor_tensor(out=lse3_sb[:], in0=lse3_sb[:], scalar=1.0,
                                        in1=bt_sb[:], op0=ALU.mult,
                                        op1=ALU.subtract).then_inc(vchain_sem, 1)
            vc += 1
            # post-AR: nll = lnS + w
            vector.wait_ge(fin_sem, 3)
            vector.wait_ge(vchain_sem, vc)
            vector.scalar_tensor_tensor(out=fin_sb[:], in0=lns_sb[:], scalar=1.0,
                                        in1=lse3_sb[:], op0=ALU.mult,
                                        op1=ALU.add).then_inc(outv_sem, 1)

    return nc


# ---------------------------------------------------------------------------
# host side


def _fp8(a, scale):
    return np.clip(np.asarray(a, np.float32) * scale, -240.0, 240.0).astype(
        ml_dtypes.float8_e4m3)


def _shard(x, y, cluster_w, cluster_b, logits_w, logits_b, cuts=CUTOFFS,
           group_cols=GROUP_COLS, mm_f=MM_F):
    x = np.asarray(x)
    y = np.asarray(y)
    cluster_w = np.asarray(cluster_w, dtype=np.float32)
    cluster_b = np.asarray(cluster_b, dtype=np.float32)
    logits_w = np.asarray(logits_w, dtype=np.float32)
    logits_b = np.asarray(logits_b, dtype=np.float32)

    xf = np.ascontiguousarray(x[:, :-1]).reshape(-1, x.shape[-1]).astype(np.float32)
    yf = y.reshape(-1).astype(np.int64)
    n = xf.shape[0]
    hid = xf.shape[1]
    ncl = len(cuts) - 1
    hg = hid // PART

    cid = np.zeros(n, dtype=np.int64)
    for i in range(1, ncl):
        cid += yf >= cuts[i]

    order = np.argsort(cid, kind="stable")
    counts = np.bincount(cid, minlength=ncl)
    bpc = [int(-(-c // PART)) for c in counts]
    nb = sum(bpc)
    ntok = nb * PART

    dev_orig = np.full(ntok, -1, dtype=np.int64)
    y_dev = np.empty(ntok, dtype=np.int64)
    cid_dev = np.empty(ntok, dtype=np.int64)
    pos = 0
    spos = 0
    for ci in range(ncl):
        cnt = int(counts[ci])
        seg = order[spos:spos + cnt]
        dev_orig[pos:pos + cnt] = seg
        y_dev[pos:pos + cnt] = yf[seg]
        y_dev[pos + cnt:pos + bpc[ci] * PART] = cuts[ci]
        cid_dev[pos:pos + bpc[ci] * PART] = ci
        pos += bpc[ci] * PART
        spos += cnt

    xf_dev = np.zeros((ntok, hid), dtype=np.float32)
    real = dev_orig >= 0
    xf_dev[real] = xf[dev_orig[real]]

    bf = ml_dtypes.bfloat16
    # fp8 DoubleRow layout: [p, g, tok] with contraction k = g*128 + p
    x8g = _fp8(xf_dev.T, SCALE_X).reshape(hg, PART, ntok)
    if USE_SWI:
        # DoubleRowSwInterleave stationary layout: per (block b, pair j):
        # sw[p, 2k+i] = x[(2j+i)*128+p, b*128 + (127-k)]
        ndr = hg // 2
        a = x8g.reshape(ndr, 2, PART, nb, PART)        # [j, i, p, b, tok]
        a = a[:, :, :, :, ::-1]                        # reverse tokens
        # -> [p, b, j, tok, i]
        a = a.transpose(2, 3, 0, 4, 1)
        x8 = np.ascontiguousarray(a.reshape(PART, nb, ndr, 2 * PART))
    else:
        x8 = np.ascontiguousarray(x8g.transpose(1, 0, 2))
    xe = np.ascontiguousarray(xf_dev).astype(bf)             # [ntok, H]
    wt = np.ascontiguousarray(logits_w.T[y_dev]).astype(bf)  # [ntok, H]

    bt = logits_b[0, y_dev].astype(np.float32).reshape(nb, PART).T.copy()
    oh = np.zeros((ntok, 3), dtype=np.float32)
    oh[np.arange(ntok), cid_dev] = 1.0
    oh = np.ascontiguousarray(oh.reshape(nb, PART, 3).transpose(1, 0, 2))

    has_bias = bool(logits_b.any() or cluster_b.any())
    widths = []
    for ci in range(ncl):
        v = cuts[ci + 1] - cuts[ci]
        assert v % N_CORES == 0
        widths.append(v // N_CORES)

    cwb = np.ascontiguousarray(np.broadcast_to(
        cluster_w.T.reshape(1, 3 * hid), (PART, 3 * hid))).astype(bf)
    clb = np.ascontiguousarray(np.broadcast_to(
        cluster_b.reshape(1, 1, 3), (PART, nb, 3))).astype(np.float32)

    w_cores = []
    brow_cores = []
    bscale = SCALE_W * SCALE_X
    for c in range(N_CORES):
        parts = []
        bparts = []
        for ci in range(ncl):
            lo = cuts[ci] + c * widths[ci]
            parts.append(logits_w[:, lo:lo + widths[ci]])
            bparts.append(logits_b[:, lo:lo + widths[ci]] * bscale)
        wc = np.concatenate(parts, 1)                       # [hid, W]
        w8 = np.ascontiguousarray(
            _fp8(wc, SCALE_W).reshape(hg, PART, -1).transpose(1, 0, 2))
        w_cores.append(w8)
        brow_cores.append(np.ascontiguousarray(np.concatenate(bparts, 1)).astype(bf))

    plan = Plan(bpc, widths, has_bias, group_cols=group_cols, hid=hid, mm_f=mm_f)

    in_maps = []
    for c in range(N_CORES):
        m = dict(x8=x8, w8=w_cores[c], xe=xe, wt=wt, oh=oh, bt=bt, cwb=cwb)
        if has_bias:
            m["brow"] = brow_cores[c]
            m["clb"] = clb
        in_maps.append(m)

    meta = dict(dev_orig=dev_orig, n=n, nb=nb)
    return plan, in_maps, meta


def _unshard(out, meta):
    nll_dev = np.ascontiguousarray(np.asarray(out, dtype=np.float32).T).reshape(-1)
    res = np.zeros(meta["n"], dtype=np.float32)
    real = meta["dev_orig"] >= 0
    res[meta["dev_orig"][real]] = nll_dev[real]
    return res


def kernel(x, y, cluster_w, cluster_b, logits_w, logits_b):
    plan, in_maps, meta = _shard(x, y, cluster_w, cluster_b, logits_w, logits_b)
    nc = build_graph(plan)
    res = run_bass_kernel_spmd(nc, in_maps, list(range(N_CORES)))
    return _unshard(res.results[0]["out"], meta)


# revision 41
# speedup vs baseline: 1.3052x; 1.3052x over previous
"""Adaptive-softmax NLL loss on 8 Trainium2 NeuronCores.

Algorithm (cluster-sparse): per token only its own cluster's log-softmax
matters, so
    nll[t] = -( cl[t, c(t)] - LSE(cl[t,:]) + logit[t, y_t] - ln S[t] )
with  S[t] = sum_{j in cluster(y_t)} exp(x_t . W[:,j] + b_j).

Sharding: tokens are cluster-sorted into 128-row blocks; each cluster's
vocab range is split evenly across the 8 cores (tensor parallel over
vocab).  Every core computes partial S for all tokens over its vocab
slice (fp8 DoubleRow matmul -> ScalarE exp with free-axis accumulate),
the partials are combined with a single small AllReduce, and each core
finishes the per-token epilogue locally.  The target logit is computed
from the host-gathered columns W[:, y] as an elementwise bf16 dot on
VectorE, as is the 3-column cluster head.  fp8 inputs are pre-scaled by
powers of two on the host; the exp's built-in scale multiplier unwinds
the scaling for free.  The odd-sized tail group of each big-cluster
block is exp-summed on VectorE via a Schraudolph bit-trick to keep
ScalarE below the TensorE floor, and dummy AllReduces warm the
collective path so the real one runs at its warm latency.
"""

import numpy as np
import ml_dtypes
from contextlib import ExitStack

import concourse.bass as bass
import concourse.mybir as mybir
from concourse.bass_utils import run_bass_kernel_spmd

F32 = mybir.dt.float32
I32 = mybir.dt.int32
BF16 = mybir.dt.bfloat16
FP8 = mybir.dt.float8e4
AF = mybir.ActivationFunctionType
ALU = mybir.AluOpType
DR = mybir.MatmulPerfMode.DoubleRow
DRSW = mybir.MatmulPerfMode.DoubleRowSwInterleave
USE_SWI = False

N_CORES = 8
PART = 128
CUTOFFS = [0, 2000, 10000, 50000]
HID = 512

GROUP_COLS = 1024   # retained for the small-scale sim configs
TCAPS = [1024, 1024, 1024, 1024]   # psum tensor widths (2 banks each)
MM_F = 512          # max matmul free size (one psum bank)
SCALE_W = 2048.0    # fp8 pre-scale for weights (power of 2)
SCALE_X = 32.0      # fp8 pre-scale for activations (power of 2)

DISABLE = set()     # bisection hooks


# ---------------------------------------------------------------------------
# planning


class Plan:
    """Static schedule shared by the host sharding code and the builder."""

    def __init__(self, blocks_per_cluster, widths, has_bias, group_cols=GROUP_COLS,
                 hid=HID, mm_f=MM_F):
        assert hid % 256 == 0
        self.hg = hid // PART          # 128-row h-groups (4)
        self.ndr = hid // 256          # DoubleRow matmuls per unit (2)
        self.hid = hid
        self.has_bias = has_bias
        self.group_cols = group_cols
        self.mm_f = mm_f
        self.widths = widths                      # per-core cols per cluster
        self.bpc = blocks_per_cluster             # blocks per cluster
        self.nb = sum(blocks_per_cluster)
        self.ncl = len(widths)
        self.act_scale = 1.0 / (SCALE_W * SCALE_X)

        # per-core w column layout: [c0 | c1 | ... ] (cluster head is
        # computed on VectorE from bf16 inputs instead)
        self.w_off = []
        off = 0
        for wd in widths:
            self.w_off.append(off)
            off += wd
        self.wcols = off

        # head-split: part0 = first cluster only; part1 = all clusters
        # except the last; part2 = the big last cluster
        self.wsplit0 = sum(widths[:-1])
        self.tsplit0 = PART * sum(blocks_per_cluster[:-1])
        self.wsplit = self.w_off[-1]
        self.tsplit = PART * sum(blocks_per_cluster[:-1])

        # blocks: cluster index per block
        self.block_cluster = []
        for ci, nblk in enumerate(blocks_per_cluster):
            self.block_cluster += [ci] * nblk

        # groups: the unit of PSUM rotation.  Asymmetric psum tensors,
        # assigned round-robin (LRU); each group is one ACT exp+accum.
        if group_cols == GROUP_COLS:
            self.tcaps = list(TCAPS)
        else:                      # small-scale sim: 4 tensors of group_cols
            self.tcaps = [group_cols] * 4
        self.groups = []   # dicts: b, gi, tidx, prev_g, units[(po,wo,F)], span
        lru = list(range(len(self.tcaps)))
        last_on = [None] * len(self.tcaps)
        for b, ci in enumerate(self.block_cluster):
            V = widths[ci]
            wo0 = self.w_off[ci]
            col = 0
            gi = 0
            while col < V:
                t = lru.pop(0)
                lru.append(t)
                gsz = min(self.tcaps[t], V - col)
                units = []
                po = 0
                rem = gsz
                while rem > 0:
                    f = min(self.mm_f, rem)
                    units.append((po, wo0 + col + po, f))
                    po += f
                    rem -= f
                g = len(self.groups)
                self.groups.append(dict(b=b, gi=gi, tidx=t, prev_g=last_on[t],
                                        units=units, span=gsz))
                last_on[t] = g
                col += gsz
                gi += 1
        self.ngroups = len(self.groups)

        # fuse ACT over pairs of full-cap groups in adjacent psum quarters
        # (the psum is one contiguous tensor; consecutive tidx => contiguous
        # columns).  The odd-sized tail group of each last-cluster block is
        # emitted unfused and offloaded to VectorE (Schraudolph exp).
        self.act_instrs = []    # dicts: b, span_off, span, slot, last_g, eng, ord
        self.act_of_group = [None] * self.ngroups
        caps = self.tcaps
        g = 0
        while g < self.ngroups:
            grp = self.groups[g]
            b = grp["b"]
            fuse = False
            if g + 1 < self.ngroups:
                nxt = self.groups[g + 1]
                if (nxt["b"] == b and nxt["tidx"] == grp["tidx"] + 1
                        and grp["span"] == caps[grp["tidx"]]
                        and nxt["span"] == caps[nxt["tidx"]]):
                    fuse = True
            off = sum(caps[:grp["tidx"]])
            idx = len(self.act_instrs)
            slot = len([a for a in self.act_instrs if a["b"] == b])
            if fuse:
                span = grp["span"] + self.groups[g + 1]["span"]
                self.act_of_group[g] = idx
                self.act_of_group[g + 1] = idx
                self.act_instrs.append(dict(b=b, span_off=off, span=span,
                                            slot=slot, last_g=g + 1, eng="act"))
                g += 2
            else:
                self.act_of_group[g] = idx
                self.act_instrs.append(dict(b=b, span_off=off, span=grp["span"],
                                            slot=slot, last_g=g, eng="act"))
                g += 1
        # offload: the final (always unfused) instr of each last-cluster block
        last_of_block = {}
        for a in self.act_instrs:
            last_of_block[a["b"]] = a
        for b, a in last_of_block.items():
            if self.block_cluster[b] == self.ncl - 1 and a["span"] < max(caps):
                a["eng"] = "dve"
        # per-engine ordinals
        na = nd = 0
        for a in self.act_instrs:
            if a["eng"] == "act":
                a["ord"] = na
                na += 1
            else:
                a["ord"] = nd
                nd += 1
        self.n_act_eng = na
        self.n_dvx = nd
        # per-block list of dve-offloaded instrs
        self.dvx_of_block = {}
        for a in self.act_instrs:
            if a["eng"] == "dve":
                self.dvx_of_block.setdefault(a["b"], []).append(a)
        self.n_act = len(self.act_instrs)
        self.max_gpb = max(a["slot"] for a in self.act_instrs) + 1
        # Schraudolph constants for the DVE exp offload
        self.dve_a = self.act_scale * (2.0 ** 23) / float(np.log(2.0))
        self.dve_b = 127.0 * 2 ** 23 - 486411.0

        # first group needing part1 (middle clusters) / part2 (last cluster)
        self.first_p1_group = None
        self.first_p2_group = None
        for g, grp in enumerate(self.groups):
            ci = self.block_cluster[grp["b"]]
            if ci not in (0, self.ncl - 1) and self.first_p1_group is None:
                self.first_p1_group = g
            if ci == self.ncl - 1 and self.first_p2_group is None:
                self.first_p2_group = g
                break


def build_graph(plan: Plan):
    nc = bass.Bass()
    HG, NB, G = plan.hg, plan.nb, plan.ngroups
    NTOK = NB * PART
    W = plan.wcols
    WS, TS = plan.wsplit, plan.tsplit

    if USE_SWI:
        x8_ext = nc.declare_dram_parameter("x8", [PART, NB, plan.ndr, 2 * PART],
                                           FP8, isOutput=False)
    else:
        x8_ext = nc.declare_dram_parameter("x8", [PART, HG, NTOK], FP8,
                                           isOutput=False)
    w8_ext = nc.declare_dram_parameter("w8", [PART, HG, W], FP8, isOutput=False)
    xe_ext = nc.declare_dram_parameter("xe", [NTOK, plan.hid], BF16, isOutput=False)
    wt_ext = nc.declare_dram_parameter("wt", [NTOK, plan.hid], BF16, isOutput=False)
    cwb_ext = nc.declare_dram_parameter("cwb", [PART, 3 * plan.hid], BF16,
                                        isOutput=False)
    oh_ext = nc.declare_dram_parameter("oh", [PART, NB, 3], F32, isOutput=False)
    bt_ext = nc.declare_dram_parameter("bt", [PART, NB], F32, isOutput=False)
    if plan.has_bias:
        brow_ext = nc.declare_dram_parameter("brow", [1, W], BF16, isOutput=False)
        clb_ext = nc.declare_dram_parameter("clb", [PART, NB, 3], F32,
                                            isOutput=False)
    out_ext = nc.declare_dram_parameter("out", [PART, NB], F32, isOutput=True)

    ar_in = nc.dram_tensor("ar_in", [PART, NB], F32)
    ar_out = nc.dram_tensor("ar_out", [PART, NB], F32, addr_space="Shared")
    dm_in = nc.dram_tensor("dm_in", [PART], F32)
    dm_out = nc.dram_tensor("dm_out", [PART], F32, addr_space="Shared")

    n_p0 = 1 + (1 if plan.tsplit0 > 0 else 0) + (1 if plan.has_bias else 0)
    n_p1 = ((1 if plan.wsplit > plan.wsplit0 else 0)
            + (1 if plan.tsplit > plan.tsplit0 else 0))
    n_misc = 2                               # oh, bt

    with ExitStack() as ctx:
        w8_sb = ctx.enter_context(nc.sbuf_tensor([PART, HG, W], FP8))
        if USE_SWI:
            x8_sb = ctx.enter_context(
                nc.sbuf_tensor([PART, NB * plan.ndr * 2 * PART], FP8))
        else:
            x8_sb = ctx.enter_context(nc.sbuf_tensor([PART, HG, NTOK], FP8))
        xe_sb = ctx.enter_context(nc.sbuf_tensor([PART, 2 * plan.hid], BF16))
        wt_sb = ctx.enter_context(nc.sbuf_tensor([PART, 2 * plan.hid], BF16))
        sacc_sb = ctx.enter_context(nc.sbuf_tensor([PART, NB, plan.max_gpb], F32))
        cl_sb = ctx.enter_context(nc.sbuf_tensor([PART, NB, 3], F32))
        ecl_sb = ctx.enter_context(nc.sbuf_tensor([PART, NB, 3], F32))
        oh_sb = ctx.enter_context(nc.sbuf_tensor([PART, NB, 3], F32))
        tmp3_sb = ctx.enter_context(nc.sbuf_tensor([PART, NB, 3], F32))
        prod_sb = ctx.enter_context(nc.sbuf_tensor([PART, 8 * plan.hid], F32))
        cwb_sb = ctx.enter_context(nc.sbuf_tensor([PART, 3 * plan.hid], BF16))
        t_sb = ctx.enter_context(nc.sbuf_tensor([PART, NB], F32))
        bt_sb = ctx.enter_context(nc.sbuf_tensor([PART, NB], F32))
        s_sb = ctx.enter_context(nc.sbuf_tensor([PART, NB], F32))
        st_sb = ctx.enter_context(nc.sbuf_tensor([PART, NB], F32))
        lns_sb = ctx.enter_context(nc.sbuf_tensor([PART, NB], F32))
        se3_sb = ctx.enter_context(nc.sbuf_tensor([PART, NB], F32))
        lse3_sb = ctx.enter_context(nc.sbuf_tensor([PART, NB], F32))
        clsel_sb = ctx.enter_context(nc.sbuf_tensor([PART, NB], F32))
        fin_sb = ctx.enter_context(nc.sbuf_tensor([PART, NB], F32))
        ones_sb = ctx.enter_context(nc.sbuf_tensor([1, PART], BF16))
        brow_sb = ctx.enter_context(nc.sbuf_tensor([1, W], BF16))
        ps = ctx.enter_context(nc.psum_tensor("ps",
                                              [PART, sum(plan.tcaps)], F32))
        pbase = [sum(plan.tcaps[:i]) for i in range(len(plan.tcaps))]
        dma_w0 = ctx.enter_context(nc.semaphore("dma_w0"))
        dma_w1 = ctx.enter_context(nc.semaphore("dma_w1"))
        dma_w2 = ctx.enter_context(nc.semaphore("dma_w2"))
        dma_misc = ctx.enter_context(nc.semaphore("dma_misc"))
        dma_ep0 = ctx.enter_context(nc.semaphore("dma_ep0"))
        dma_ep1 = ctx.enter_context(nc.semaphore("dma_ep1"))
        dma_out = ctx.enter_context(nc.semaphore("dma_out"))
        mm_sem = ctx.enter_context(nc.semaphore("mm_sem"))
        act_sem = ctx.enter_context(nc.semaphore("act_sem"))
        dma_cwb = ctx.enter_context(nc.semaphore("dma_cwb"))
        tdot_sem = ctx.enter_context(nc.semaphore("tdot_sem"))
        veini_sem = ctx.enter_context(nc.semaphore("veini_sem"))
        ve_sem = ctx.enter_context(nc.semaphore("ve_sem"))
        ve2_sem = ctx.enter_context(nc.semaphore("ve2_sem"))
        cc_sem = ctx.enter_context(nc.semaphore("cc_sem"))
        fin_sem = ctx.enter_context(nc.semaphore("fin_sem"))
        outv_sem = ctx.enter_context(nc.semaphore("outv_sem"))
        vchain_sem = ctx.enter_context(nc.semaphore("vchain_sem"))
        gp_sem = ctx.enter_context(nc.semaphore("gp_sem"))
        dvx_sem = ctx.enter_context(nc.semaphore("dvx_sem"))
        dvxp_sem = ctx.enter_context(nc.semaphore("dvxp_sem"))
        block = ctx.enter_context(nc.Block())

        WS0, TS0 = plan.wsplit0, plan.tsplit0

        @block.sync
        def _(sync):
            # part 0: just the first cluster's slice, to start PE asap
            sync.dma_start(out=w8_sb[:, :, 0:WS0],
                           in_=w8_ext[:, :, 0:WS0]).then_inc(dma_w0, 16)
            # part 1: remaining small clusters (empty when no middle part)
            if WS > WS0:
                sync.dma_start(out=w8_sb[:, :, WS0:WS],
                               in_=w8_ext[:, :, WS0:WS]).then_inc(dma_w1, 16)

            if plan.has_bias:
                sync.dma_start(out=brow_sb[:], in_=brow_ext[:]).then_inc(dma_w0, 16)
            sync.dma_start(out=cwb_sb[:], in_=cwb_ext[:]).then_inc(dma_cwb, 16)
            if plan.has_bias:
                sync.dma_start(out=tmp3_sb[:], in_=clb_ext[:]).then_inc(dma_cwb, 16)
            # part 2: the big cluster
            sync.dma_start(out=w8_sb[:, :, WS:W],
                           in_=w8_ext[:, :, WS:W]).then_inc(dma_w2, 16)

            # misc for the epilogue
            sync.dma_start(out=oh_sb[:], in_=oh_ext[:]).then_inc(dma_misc, 16)
            sync.dma_start(out=bt_sb[:], in_=bt_ext[:]).then_inc(dma_misc, 16)
            # epilogue tiles, double-buffered, paced by the t-dot consumer
            for e in range(NB):
                if e >= 2:
                    sync.wait_ge(tdot_sem, 4 * (e - 1))
                sem_e = dma_ep0 if e % 2 == 0 else dma_ep1
                toff = (e % 2) * plan.hid
                sync.dma_start(out=xe_sb[:, toff:toff + plan.hid],
                               in_=xe_ext[e * PART:(e + 1) * PART, :]
                               ).then_inc(sem_e, 16)
                sync.dma_start(out=wt_sb[:, toff:toff + plan.hid],
                               in_=wt_ext[e * PART:(e + 1) * PART, :]
                               ).then_inc(sem_e, 16)
            # S partials out, AllReduce result back, final output
            sync.wait_ge(ve_sem, 1)
            sync.dma_start(out=ar_in[:], in_=s_sb[:]).then_inc(dma_out, 16)
            sync.wait_ge(cc_sem, 3)
            sync.dma_start(out=st_sb[:], in_=ar_out[:]).then_inc(dma_out, 16)
            sync.wait_ge(outv_sem, 1)
            sync.dma_start(out=out_ext[:], in_=fin_sb[:]).then_inc(dma_out, 16)

        @block.gpsimd
        def _(gpsimd):
            # tiny dummy collective issued immediately: pays the cold-start
            # and entry-barrier cost concurrently with the main compute, so
            # the real AllReduce at the end runs on a warm path
            gpsimd.dma_start(out=dm_in[:],
                             in_=bt_ext[:].rearrange("p e -> (p e)")[0:PART]
                             ).then_inc(gp_sem, 16)
            gpsimd.wait_ge(gp_sem, 16)
            gpsimd.collective_compute(
                "AllReduce",
                ALU.add,
                ins=[dm_in[:]],
                outs=[dm_out[:]],
                replica_groups=[list(range(N_CORES))],
            ).then_inc(cc_sem, 1)
            gpsimd.wait_ge(mm_sem, (G * 11) // 20)
            gpsimd.collective_compute(
                "AllReduce",
                ALU.add,
                ins=[dm_in[:]],
                outs=[dm_out[:]],
                replica_groups=[list(range(N_CORES))],
            ).then_inc(cc_sem, 1)
            gpsimd.wait_ge(dma_out, 16)
            gpsimd.collective_compute(
                "AllReduce",
                ALU.add,
                ins=[ar_in[:]],
                outs=[ar_out[:]],
                replica_groups=[list(range(N_CORES))],
            ).then_inc(cc_sem, 1)

        @block.tensor
        def _(tensor):
            tensor.wait_ge(dma_w0, 16 * n_p0)
            if plan.has_bias:
                tensor.wait_ge(veini_sem, 2)  # ones row ready
            for g, grp in enumerate(plan.groups):
                pb0 = pbase[grp["tidx"]]
                if g == plan.first_p1_group and n_p1 > 0:
                    tensor.wait_ge(dma_w1, 16 * n_p1)
                if g == plan.first_p2_group:
                    tensor.wait_ge(dma_w2, 32)
                if grp["prev_g"] is not None:
                    pa = plan.act_instrs[plan.act_of_group[grp["prev_g"]]]
                    tensor.wait_ge(act_sem if pa["eng"] == "act" else dvx_sem,
                                   pa["ord"] + 1)
                b = grp["b"]
                nunits = len(grp["units"])
                for ui, (po, wo, f) in enumerate(grp["units"]):
                    for j in range(plan.ndr):
                        if USE_SWI:
                            xoff = (b * plan.ndr + j) * 2 * PART
                            lhsT = x8_sb[:, xoff:xoff + 2 * PART]
                        else:
                            lhsT = x8_sb[:, 2 * j:2 * j + 2,
                                         b * PART:(b + 1) * PART]
                        mm = tensor.matmul(
                            ps[:, pb0 + po:pb0 + po + f],
                            lhsT=lhsT,
                            rhs=w8_sb[:, 2 * j:2 * j + 2, wo:wo + f],
                            start=(j == 0),
                            stop=(j == plan.ndr - 1 and not plan.has_bias),
                            perf_mode=DRSW if USE_SWI else DR)
                        if (j == plan.ndr - 1 and not plan.has_bias
                                and ui == nunits - 1):
                            mm.then_inc(mm_sem, 1)
                    if plan.has_bias:
                        mm = tensor.matmul(
                            ps[:, pb0 + po:pb0 + po + f],
                            lhsT=ones_sb[:],
                            rhs=brow_sb[0:1, wo:wo + f],
                            start=False, stop=True)
                        if ui == nunits - 1:
                            mm.then_inc(mm_sem, 1)

        @block.scalar
        def _(scalar):
            # x8 loads ride the ACT engine's parallel HWDGE ring
            def x8_dma(sem, tok_lo, tok_hi):
                if USE_SWI:
                    blo, bhi = tok_lo // PART, tok_hi // PART
                    clo, chi = blo * plan.ndr * 2 * PART, bhi * plan.ndr * 2 * PART
                    scalar.dma_start(
                        out=x8_sb[:, clo:chi],
                        in_=x8_ext[:, blo:bhi, :, :]).then_inc(sem, 16)
                else:
                    scalar.dma_start(out=x8_sb[:, :, tok_lo:tok_hi],
                                     in_=x8_ext[:, :, tok_lo:tok_hi]).then_inc(sem, 16)

            if TS0 > 0:
                x8_dma(dma_w0, 0, TS0)
            if TS > TS0:
                x8_dma(dma_w1, TS0, TS)
            x8_dma(dma_w2, TS, NTOK)
            scalar.wait_ge(veini_sem, 1)
            for a in plan.act_instrs:
                if a["eng"] != "act":
                    continue
                scalar.wait_ge(mm_sem, a["last_g"] + 1)
                o, sp = a["span_off"], a["span"]
                scalar.activation(
                    ps[:, o:o + sp],
                    ps[:, o:o + sp],
                    AF.Exp,
                    scale=plan.act_scale,
                    accum_out=sacc_sb[:, a["b"], a["slot"]:a["slot"] + 1],
                ).then_inc(act_sem, 1)
            # epilogue
            if plan.has_bias:
                scalar.wait_ge(ve2_sem, 2)
            else:
                scalar.wait_ge(tdot_sem, 4 * NB)
            scalar.activation(ecl_sb[:], cl_sb[:], AF.Exp).then_inc(fin_sem, 1)
            scalar.wait_ge(ve2_sem, 3 if plan.has_bias else 1)
            scalar.activation(lse3_sb[:], se3_sb[:], AF.Ln).then_inc(fin_sem, 1)
            scalar.wait_ge(dma_out, 32)
            scalar.activation(lns_sb[:], st_sb[:], AF.Ln).then_inc(fin_sem, 1)

        @block.vector
        def _(vector):
            vector.memset(sacc_sb[:], 0.0).then_inc(veini_sem, 1)
            if plan.has_bias:
                vector.memset(ones_sb[:], 1.0).then_inc(veini_sem, 1)
            vector.wait_ge(dma_cwb, 32 if plan.has_bias else 16)
            H = plan.hid
            for b in range(NB):
                # Schraudolph exp+sum for the previous block's offloaded tail
                # group goes first so its psum quarter frees as soon as the
                # matmuls finish
                for a in plan.dvx_of_block.get(b - 1, []):
                    o, sp = a["span_off"], a["span"]
                    vector.wait_ge(mm_sem, a["last_g"] + 1)
                    vector.tensor_scalar(
                        out=ps[:, o:o + sp].bitcast(I32),
                        in0=ps[:, o:o + sp],
                        scalar1=plan.dve_a,
                        scalar2=plan.dve_b,
                        op0=ALU.mult,
                        op1=ALU.add).then_inc(dvxp_sem, 1)
                    vector.wait_ge(dvxp_sem, a["ord"] + 1)
                    vector.reduce_sum(
                        sacc_sb[:, a["b"], a["slot"]:a["slot"] + 1],
                        ps[:, o:o + sp],
                        axis=mybir.AxisListType.X).then_inc(dvx_sem, 1)
                vector.wait_ge(dma_ep0 if b % 2 == 0 else dma_ep1,
                               32 * (b // 2 + 1))
                toff = (b % 2) * H
                # target-logit dot + 3 cluster-head dots, each with its own
                # scratch slot (WAW across tiles is ordered transitively via
                # the DMA pacing)
                po = (b % 2) * 4 * H
                vector.scalar_tensor_tensor(
                    out=prod_sb[:, po:po + H],
                    in0=xe_sb[:, toff:toff + H],
                    scalar=1.0,
                    in1=wt_sb[:, toff:toff + H],
                    op0=ALU.mult,
                    op1=ALU.mult,
                    accum_out=t_sb[:, b:b + 1],
                ).then_inc(tdot_sem, 1)
                for i in range(3):
                    vector.scalar_tensor_tensor(
                        out=prod_sb[:, po + (i + 1) * H:po + (i + 2) * H],
                        in0=xe_sb[:, toff:toff + H],
                        scalar=1.0,
                        in1=cwb_sb[:, i * H:(i + 1) * H],
                        op0=ALU.mult,
                        op1=ALU.mult,
                        accum_out=cl_sb[:, b, i:i + 1],
                    ).then_inc(tdot_sem, 1)
            for a in plan.dvx_of_block.get(NB - 1, []):
                o, sp = a["span_off"], a["span"]
                vector.wait_ge(mm_sem, a["last_g"] + 1)
                vector.tensor_scalar(
                    out=ps[:, o:o + sp].bitcast(I32),
                    in0=ps[:, o:o + sp],
                    scalar1=plan.dve_a,
                    scalar2=plan.dve_b,
                    op0=ALU.mult,
                    op1=ALU.add).then_inc(dvxp_sem, 1)
                vector.wait_ge(dvxp_sem, a["ord"] + 1)
                vector.reduce_sum(
                    sacc_sb[:, a["b"], a["slot"]:a["slot"] + 1],
                    ps[:, o:o + sp],
                    axis=mybir.AxisListType.X).then_inc(dvx_sem, 1)
            # ---- tail (serialized through vchain_sem for the race detector)
            vc = 0
            if plan.has_bias:
                # cl += cluster_b (clb staged in tmp3_sb)
                vector.wait_ge(tdot_sem, 4 * NB)
                vector.wait_ge(dma_cwb, 32)
                vector.tensor_tensor(cl_sb[:], cl_sb[:], tmp3_sb[:],
                                     ALU.add).then_inc(ve2_sem, 2)
            vector.wait_ge(act_sem, plan.n_act_eng)
            vector.wait_ge(dvx_sem, plan.n_dvx)
            vector.tensor_reduce(s_sb[:], sacc_sb[:], mybir.AxisListType.X,
                                 ALU.add).then_inc(ve_sem, 1)
            # cluster-head select (overlaps the AllReduce)
            vector.wait_ge(dma_misc, 16 * n_misc)
            if plan.has_bias:
                vector.wait_ge(ve2_sem, 2)
            else:
                vector.wait_ge(tdot_sem, 4 * NB)
            vector.tensor_tensor(tmp3_sb[:], cl_sb[:], oh_sb[:],
                                 ALU.mult).then_inc(vchain_sem, 1)
            vc += 1
            vector.wait_ge(vchain_sem, vc)
            vector.tensor_reduce(clsel_sb[:], tmp3_sb[:], mybir.AxisListType.X,
                                 ALU.add).then_inc(vchain_sem, 1)
            vc += 1
            vector.wait_ge(fin_sem, 1)
            vector.tensor_reduce(se3_sb[:], ecl_sb[:], mybir.AxisListType.X,
                                 ALU.add).then_inc(ve2_sem, 1)
            # pre-AR: w = lse3 - clsel - t - bt  (staged in lse3_sb)
            vector.wait_ge(fin_sem, 2)
            vector.scalar_tensor_tensor(out=lse3_sb[:], in0=lse3_sb[:], scalar=1.0,
                                        in1=clsel_sb[:], op0=ALU.mult,
                                        op1=ALU.subtract).then_inc(vchain_sem, 1)
            vc += 1
            vector.wait_ge(vchain_sem, vc)
            vector.scalar_tensor_tensor(out=lse3_sb[:], in0=lse3_sb[:], scalar=1.0,
                                        in1=t_sb[:], op0=ALU.mult,
                                        op1=ALU.subtract).then_inc(vchain_sem, 1)
            vc += 1
            vector.wait_ge(vchain_sem, vc)
            vector.scalar_tensor_tensor(out=lse3_sb[:], in0=lse3_sb[:], scalar=1.0,
                                        in1=bt_sb[:], op0=ALU.mult,
                                        op1=ALU.subtract).then_inc(vchain_sem, 1)
            vc += 1
            # post-AR: nll = lnS + w
            vector.wait_ge(fin_sem, 3)
            vector.wait_ge(vchain_sem, vc)
            vector.scalar_tensor_tensor(out=fin_sb[:], in0=lns_sb[:], scalar=1.0,
                                        in1=lse3_sb[:], op0=ALU.mult,
                                        op1=ALU.add).then_inc(outv_sem, 1)

    return nc


# ---------------------------------------------------------------------------
# host side


def _fp8(a, scale):
    return np.clip(np.asarray(a, np.float32) * scale, -240.0, 240.0).astype(
        ml_dtypes.float8_e4m3)


def _shard(x, y, cluster_w, cluster_b, logits_w, logits_b, cuts=CUTOFFS,
           group_cols=GROUP_COLS, mm_f=MM_F):
    x = np.asarray(x)
    y = np.asarray(y)
    cluster_w = np.asarray(cluster_w, dtype=np.float32)
    cluster_b = np.asarray(cluster_b, dtype=np.float32)
    logits_w = np.asarray(logits_w, dtype=np.float32)
    logits_b = np.asarray(logits_b, dtype=np.float32)

    xf = np.ascontiguousarray(x[:, :-1]).reshape(-1, x.shape[-1]).astype(np.float32)
    yf = y.reshape(-1).astype(np.int64)
    n = xf.shape[0]
    hid = xf.shape[1]
    ncl = len(cuts) - 1
    hg = hid // PART

    cid = np.zeros(n, dtype=np.int64)
    for i in range(1, ncl):
        cid += yf >= cuts[i]

    order = np.argsort(cid, kind="stable")
    counts = np.bincount(cid, minlength=ncl)
    bpc = [int(-(-c // PART)) for c in counts]
    nb = sum(bpc)
    ntok = nb * PART

    dev_orig = np.full(ntok, -1, dtype=np.int64)
    y_dev = np.empty(ntok, dtype=np.int64)
    cid_dev = np.empty(ntok, dtype=np.int64)
    pos = 0
    spos = 0
    for ci in range(ncl):
        cnt = int(counts[ci])
        seg = order[spos:spos + cnt]
        dev_orig[pos:pos + cnt] = seg
        y_dev[pos:pos + cnt] = yf[seg]
        y_dev[pos + cnt:pos + bpc[ci] * PART] = cuts[ci]
        cid_dev[pos:pos + bpc[ci] * PART] = ci
        pos += bpc[ci] * PART
        spos += cnt

    xf_dev = np.zeros((ntok, hid), dtype=np.float32)
    real = dev_orig >= 0
    xf_dev[real] = xf[dev_orig[real]]

    bf = ml_dtypes.bfloat16
    # fp8 DoubleRow layout: [p, g, tok] with contraction k = g*128 + p
    x8g = _fp8(xf_dev.T, SCALE_X).reshape(hg, PART, ntok)
    if USE_SWI:
        # DoubleRowSwInterleave stationary layout: per (block b, pair j):
        # sw[p, 2k+i] = x[(2j+i)*128+p, b*128 + (127-k)]
        ndr = hg // 2
        a = x8g.reshape(ndr, 2, PART, nb, PART)        # [j, i, p, b, tok]
        a = a[:, :, :, :, ::-1]                        # reverse tokens
        # -> [p, b, j, tok, i]
        a = a.transpose(2, 3, 0, 4, 1)
        x8 = np.ascontiguousarray(a.reshape(PART, nb, ndr, 2 * PART))
    else:
        x8 = np.ascontiguousarray(x8g.transpose(1, 0, 2))
    xe = np.ascontiguousarray(xf_dev).astype(bf)             # [ntok, H]
    wt = np.ascontiguousarray(logits_w.T[y_dev]).astype(bf)  # [ntok, H]

    bt = logits_b[0, y_dev].astype(np.float32).reshape(nb, PART).T.copy()
    oh = np.zeros((ntok, 3), dtype=np.float32)
    oh[np.arange(ntok), cid_dev] = 1.0
    oh = np.ascontiguousarray(oh.reshape(nb, PART, 3).transpose(1, 0, 2))

    has_bias = bool(logits_b.any() or cluster_b.any())
    widths = []
    for ci in range(ncl):
        v = cuts[ci + 1] - cuts[ci]
        assert v % N_CORES == 0
        widths.append(v // N_CORES)

    cwb = np.ascontiguousarray(np.broadcast_to(
        cluster_w.T.reshape(1, 3 * hid), (PART, 3 * hid))).astype(bf)
    clb = np.ascontiguousarray(np.broadcast_to(
        cluster_b.reshape(1, 1, 3), (PART, nb, 3))).astype(np.float32)

    w_cores = []
    brow_cores = []
    bscale = SCALE_W * SCALE_X
    for c in range(N_CORES):
        parts = []
        bparts = []
        for ci in range(ncl):
            lo = cuts[ci] + c * widths[ci]
            parts.append(logits_w[:, lo:lo + widths[ci]])
            bparts.append(logits_b[:, lo:lo + widths[ci]] * bscale)
        wc = np.concatenate(parts, 1)                       # [hid, W]
        w8 = np.ascontiguousarray(
            _fp8(wc, SCALE_W).reshape(hg, PART, -1).transpose(1, 0, 2))
        w_cores.append(w8)
        brow_cores.append(np.ascontiguousarray(np.concatenate(bparts, 1)).astype(bf))

    plan = Plan(bpc, widths, has_bias, group_cols=group_cols, hid=hid, mm_f=mm_f)

    in_maps = []
    for c in range(N_CORES):
        m = dict(x8=x8, w8=w_cores[c], xe=xe, wt=wt, oh=oh, bt=bt, cwb=cwb)
        if has_bias:
            m["brow"] = brow_cores[c]
            m["clb"] = clb
        in_maps.append(m)

    meta = dict(dev_orig=dev_orig, n=n, nb=nb)
    return plan, in_maps, meta


def _unshard(out, meta):
    nll_dev = np.ascontiguousarray(np.asarray(out, dtype=np.float32).T).reshape(-1)
    res = np.zeros(meta["n"], dtype=np.float32)
    real = meta["dev_orig"] >= 0
    res[meta["dev_orig"][real]] = nll_dev[real]
    return res


def kernel(x, y, cluster_w, cluster_b, logits_w, logits_b):
    plan, in_maps, meta = _shard(x, y, cluster_w, cluster_b, logits_w, logits_b)
    nc = build_graph(plan)
    res = run_bass_kernel_spmd(nc, in_maps, list(range(N_CORES)))
    return _unshard(res.results[0]["out"], meta)


# revision 43
# speedup vs baseline: 1.4115x; 1.0815x over previous
"""Adaptive-softmax NLL loss on 8 Trainium2 NeuronCores.

Algorithm (cluster-sparse): per token only its own cluster's log-softmax
matters, so
    nll[t] = -( cl[t, c(t)] - LSE(cl[t,:]) + logit[t, y_t] - ln S[t] )
with  S[t] = sum_{j in cluster(y_t)} exp(x_t . W[:,j] + b_j).

Sharding: tokens are cluster-sorted into 128-row blocks; each cluster's
vocab range is split evenly across the 8 cores (tensor parallel over
vocab).  Every core computes partial S for all tokens over its vocab
slice (fp8 DoubleRow matmul -> ScalarE exp with free-axis accumulate),
the partials are combined with a single small AllReduce, and each core
finishes the per-token epilogue locally.  The target logit is computed
from the host-gathered columns W[:, y] as an elementwise bf16 dot on
VectorE, as is the 3-column cluster head.  fp8 inputs are pre-scaled by
powers of two on the host; the exp's built-in scale multiplier unwinds
the scaling for free.  The odd-sized tail group of each big-cluster
block is exp-summed on VectorE via a Schraudolph bit-trick to keep
ScalarE below the TensorE floor, and dummy AllReduces warm the
collective path so the real one runs at its warm latency.
"""

import numpy as np
import ml_dtypes
from contextlib import ExitStack

import concourse.bass as bass
import concourse.mybir as mybir
from concourse.bass_utils import run_bass_kernel_spmd

F32 = mybir.dt.float32
I32 = mybir.dt.int32
BF16 = mybir.dt.bfloat16
FP8 = mybir.dt.float8e4
AF = mybir.ActivationFunctionType
ALU = mybir.AluOpType
DR = mybir.MatmulPerfMode.DoubleRow
DRSW = mybir.MatmulPerfMode.DoubleRowSwInterleave
USE_SWI = False

N_CORES = 8
PART = 128
CUTOFFS = [0, 2000, 10000, 50000]
HID = 512

GROUP_COLS = 1024   # retained for the small-scale sim configs
TCAPS = [1024, 1024, 1024, 1024]   # psum tensor widths (2 banks each)
MM_F = 512          # max matmul free size (one psum bank)
SCALE_W = 2048.0    # fp8 pre-scale for weights (power of 2)
SCALE_X = 32.0      # fp8 pre-scale for activations (power of 2)

DISABLE = set()     # bisection hooks


# ---------------------------------------------------------------------------
# planning


class Plan:
    """Static schedule shared by the host sharding code and the builder."""

    def __init__(self, blocks_per_cluster, widths, has_bias, group_cols=GROUP_COLS,
                 hid=HID, mm_f=MM_F):
        assert hid % 256 == 0
        self.hg = hid // PART          # 128-row h-groups (4)
        self.ndr = hid // 256          # DoubleRow matmuls per unit (2)
        self.hid = hid
        self.has_bias = has_bias
        self.group_cols = group_cols
        self.mm_f = mm_f
        self.widths = widths                      # per-core cols per cluster
        self.bpc = blocks_per_cluster             # blocks per cluster
        self.nb = sum(blocks_per_cluster)
        self.ncl = len(widths)
        self.act_scale = 1.0 / (SCALE_W * SCALE_X)

        # per-core w column layout: [c0 | c1 | ... ] (cluster head is
        # computed on VectorE from bf16 inputs instead)
        self.w_off = []
        off = 0
        for wd in widths:
            self.w_off.append(off)
            off += wd
        self.wcols = off

        # head-split: part0 = first cluster only; part1 = all clusters
        # except the last; part2 = the big last cluster
        self.wsplit0 = sum(widths[:-1])
        self.tsplit0 = PART * sum(blocks_per_cluster[:-1])
        self.wsplit = self.w_off[-1]
        self.tsplit = PART * sum(blocks_per_cluster[:-1])

        # blocks: cluster index per block
        self.block_cluster = []
        for ci, nblk in enumerate(blocks_per_cluster):
            self.block_cluster += [ci] * nblk

        # groups: the unit of PSUM rotation.  Asymmetric psum tensors,
        # assigned round-robin (LRU); each group is one ACT exp+accum.
        if group_cols == GROUP_COLS:
            self.tcaps = list(TCAPS)
        else:                      # small-scale sim: 4 tensors of group_cols
            self.tcaps = [group_cols] * 4
        self.groups = []   # dicts: b, gi, tidx, prev_g, units[(po,wo,F)], span
        lru = list(range(len(self.tcaps)))
        last_on = [None] * len(self.tcaps)
        for b, ci in enumerate(self.block_cluster):
            V = widths[ci]
            wo0 = self.w_off[ci]
            col = 0
            gi = 0
            while col < V:
                t = lru.pop(0)
                lru.append(t)
                gsz = min(self.tcaps[t], V - col)
                units = []
                po = 0
                rem = gsz
                while rem > 0:
                    f = min(self.mm_f, rem)
                    units.append((po, wo0 + col + po, f))
                    po += f
                    rem -= f
                g = len(self.groups)
                self.groups.append(dict(b=b, gi=gi, tidx=t, prev_g=last_on[t],
                                        units=units, span=gsz))
                last_on[t] = g
                col += gsz
                gi += 1
        self.ngroups = len(self.groups)

        # fuse ACT over pairs of full-cap groups in adjacent psum quarters
        # (the psum is one contiguous tensor; consecutive tidx => contiguous
        # columns).  The odd-sized tail group of each last-cluster block is
        # emitted unfused and offloaded to VectorE (Schraudolph exp).
        self.act_instrs = []    # dicts: b, span_off, span, slot, last_g, eng, ord
        self.act_of_group = [None] * self.ngroups
        caps = self.tcaps
        g = 0
        while g < self.ngroups:
            grp = self.groups[g]
            b = grp["b"]
            fuse = False
            if g + 1 < self.ngroups:
                nxt = self.groups[g + 1]
                if (nxt["b"] == b and nxt["tidx"] == grp["tidx"] + 1
                        and grp["span"] == caps[grp["tidx"]]
                        and nxt["span"] == caps[nxt["tidx"]]):
                    fuse = True
            off = sum(caps[:grp["tidx"]])
            idx = len(self.act_instrs)
            slot = len([a for a in self.act_instrs if a["b"] == b])
            if fuse:
                span = grp["span"] + self.groups[g + 1]["span"]
                self.act_of_group[g] = idx
                self.act_of_group[g + 1] = idx
                self.act_instrs.append(dict(b=b, span_off=off, span=span,
                                            slot=slot, last_g=g + 1, eng="act"))
                g += 2
            else:
                self.act_of_group[g] = idx
                self.act_instrs.append(dict(b=b, span_off=off, span=grp["span"],
                                            slot=slot, last_g=g, eng="act"))
                g += 1
        # offload: the final (always unfused) instr of each last-cluster block
        last_of_block = {}
        for a in self.act_instrs:
            last_of_block[a["b"]] = a
        for b, a in last_of_block.items():
            if self.block_cluster[b] == self.ncl - 1 and a["span"] < max(caps):
                a["eng"] = "dve"
        # per-engine ordinals
        na = nd = 0
        for a in self.act_instrs:
            if a["eng"] == "act":
                a["ord"] = na
                na += 1
            else:
                a["ord"] = nd
                nd += 1
        self.n_act_eng = na
        self.n_dvx = nd
        # per-block list of dve-offloaded instrs
        self.dvx_of_block = {}
        for a in self.act_instrs:
            if a["eng"] == "dve":
                self.dvx_of_block.setdefault(a["b"], []).append(a)
        self.n_act = len(self.act_instrs)
        self.max_gpb = max(a["slot"] for a in self.act_instrs) + 1
        # Schraudolph constants for the DVE exp offload
        self.dve_a = self.act_scale * (2.0 ** 23) / float(np.log(2.0))
        self.dve_b = 127.0 * 2 ** 23 - 486411.0

        # first group needing part1 (middle clusters) / part2 (last cluster)
        self.first_p1_group = None
        self.first_p2_group = None
        for g, grp in enumerate(self.groups):
            ci = self.block_cluster[grp["b"]]
            if ci not in (0, self.ncl - 1) and self.first_p1_group is None:
                self.first_p1_group = g
            if ci == self.ncl - 1 and self.first_p2_group is None:
                self.first_p2_group = g
                break


def build_graph(plan: Plan):
    nc = bass.Bass()
    HG, NB, G = plan.hg, plan.nb, plan.ngroups
    NTOK = NB * PART
    W = plan.wcols
    WS, TS = plan.wsplit, plan.tsplit

    if USE_SWI:
        x8_ext = nc.declare_dram_parameter("x8", [PART, NB, plan.ndr, 2 * PART],
                                           FP8, isOutput=False)
    else:
        x8_ext = nc.declare_dram_parameter("x8", [PART, HG, NTOK], FP8,
                                           isOutput=False)
    w8_ext = nc.declare_dram_parameter("w8", [PART, HG, W], FP8, isOutput=False)
    xe_ext = nc.declare_dram_parameter("xe", [NTOK, plan.hid], BF16, isOutput=False)
    wt_ext = nc.declare_dram_parameter("wt", [NTOK, plan.hid], BF16, isOutput=False)
    cwb_ext = nc.declare_dram_parameter("cwb", [PART, 3 * plan.hid], BF16,
                                        isOutput=False)
    oh_ext = nc.declare_dram_parameter("oh", [PART, NB, 3], F32, isOutput=False)
    bt_ext = nc.declare_dram_parameter("bt", [PART, NB], F32, isOutput=False)
    if plan.has_bias:
        brow_ext = nc.declare_dram_parameter("brow", [1, W], BF16, isOutput=False)
        clb_ext = nc.declare_dram_parameter("clb", [PART, NB, 3], F32,
                                            isOutput=False)
    out_ext = nc.declare_dram_parameter("out", [PART, NB], F32, isOutput=True)

    ar_in = nc.dram_tensor("ar_in", [PART, NB], F32)
    ar_out = nc.dram_tensor("ar_out", [PART, NB], F32, addr_space="Shared")
    dm_in = nc.dram_tensor("dm_in", [PART], F32)
    dm_out = nc.dram_tensor("dm_out", [PART], F32, addr_space="Shared")

    n_p0 = 1 + (1 if plan.tsplit0 > 0 else 0) + (1 if plan.has_bias else 0)
    n_p1 = ((1 if plan.wsplit > plan.wsplit0 else 0)
            + (1 if plan.tsplit > plan.tsplit0 else 0))
    n_misc = 2                               # oh, bt

    with ExitStack() as ctx:
        w8_sb = ctx.enter_context(nc.sbuf_tensor([PART, HG, W], FP8))
        if USE_SWI:
            x8_sb = ctx.enter_context(
                nc.sbuf_tensor([PART, NB * plan.ndr * 2 * PART], FP8))
        else:
            x8_sb = ctx.enter_context(nc.sbuf_tensor([PART, HG, NTOK], FP8))
        xe_sb = ctx.enter_context(nc.sbuf_tensor([PART, 2 * plan.hid], BF16))
        wt_sb = ctx.enter_context(nc.sbuf_tensor([PART, 2 * plan.hid], BF16))
        sacc_sb = ctx.enter_context(nc.sbuf_tensor([PART, NB, plan.max_gpb], F32))
        cl_sb = ctx.enter_context(nc.sbuf_tensor([PART, NB, 3], F32))
        ecl_sb = ctx.enter_context(nc.sbuf_tensor([PART, NB, 3], F32))
        oh_sb = ctx.enter_context(nc.sbuf_tensor([PART, NB, 3], F32))
        tmp3_sb = ctx.enter_context(nc.sbuf_tensor([PART, NB, 3], F32))
        prod_sb = ctx.enter_context(nc.sbuf_tensor([PART, 8 * plan.hid], F32))
        cwb_sb = ctx.enter_context(nc.sbuf_tensor([PART, 3 * plan.hid], BF16))
        t_sb = ctx.enter_context(nc.sbuf_tensor([PART, NB], F32))
        bt_sb = ctx.enter_context(nc.sbuf_tensor([PART, NB], F32))
        s_sb = ctx.enter_context(nc.sbuf_tensor([PART, NB], F32))
        st_sb = ctx.enter_context(nc.sbuf_tensor([PART, NB], F32))
        lns_sb = ctx.enter_context(nc.sbuf_tensor([PART, NB], F32))
        se3_sb = ctx.enter_context(nc.sbuf_tensor([PART, NB], F32))
        lse3_sb = ctx.enter_context(nc.sbuf_tensor([PART, NB], F32))
        clsel_sb = ctx.enter_context(nc.sbuf_tensor([PART, NB], F32))
        fin_sb = ctx.enter_context(nc.sbuf_tensor([PART, NB], F32))
        ones_sb = ctx.enter_context(nc.sbuf_tensor([1, PART], BF16))
        brow_sb = ctx.enter_context(nc.sbuf_tensor([1, W], BF16))
        ps = ctx.enter_context(nc.psum_tensor("ps",
                                              [PART, sum(plan.tcaps)], F32))
        pbase = [sum(plan.tcaps[:i]) for i in range(len(plan.tcaps))]
        dma_w0 = ctx.enter_context(nc.semaphore("dma_w0"))
        dma_w1 = ctx.enter_context(nc.semaphore("dma_w1"))
        dma_w2 = ctx.enter_context(nc.semaphore("dma_w2"))
        dma_misc = ctx.enter_context(nc.semaphore("dma_misc"))
        dma_ep0 = ctx.enter_context(nc.semaphore("dma_ep0"))
        dma_ep1 = ctx.enter_context(nc.semaphore("dma_ep1"))
        dma_out = ctx.enter_context(nc.semaphore("dma_out"))
        mm_sem = ctx.enter_context(nc.semaphore("mm_sem"))
        act_sem = ctx.enter_context(nc.semaphore("act_sem"))
        dma_cwb = ctx.enter_context(nc.semaphore("dma_cwb"))
        tdot_sem = ctx.enter_context(nc.semaphore("tdot_sem"))
        veini_sem = ctx.enter_context(nc.semaphore("veini_sem"))
        ve_sem = ctx.enter_context(nc.semaphore("ve_sem"))
        ve2_sem = ctx.enter_context(nc.semaphore("ve2_sem"))
        cc_sem = ctx.enter_context(nc.semaphore("cc_sem"))
        fin_sem = ctx.enter_context(nc.semaphore("fin_sem"))
        outv_sem = ctx.enter_context(nc.semaphore("outv_sem"))
        vchain_sem = ctx.enter_context(nc.semaphore("vchain_sem"))
        gp_sem = ctx.enter_context(nc.semaphore("gp_sem"))
        dvx_sem = ctx.enter_context(nc.semaphore("dvx_sem"))
        dvxp_sem = ctx.enter_context(nc.semaphore("dvxp_sem"))
        block = ctx.enter_context(nc.Block())

        WS0, TS0 = plan.wsplit0, plan.tsplit0

        @block.sync
        def _(sync):
            # part 0: just the first cluster's slice, to start PE asap
            sync.dma_start(out=w8_sb[:, :, 0:WS0],
                           in_=w8_ext[:, :, 0:WS0]).then_inc(dma_w0, 16)
            # part 1: remaining small clusters (empty when no middle part)
            if WS > WS0:
                sync.dma_start(out=w8_sb[:, :, WS0:WS],
                               in_=w8_ext[:, :, WS0:WS]).then_inc(dma_w1, 16)

            if plan.has_bias:
                sync.dma_start(out=brow_sb[:], in_=brow_ext[:]).then_inc(dma_w0, 16)
            sync.dma_start(out=cwb_sb[:], in_=cwb_ext[:]).then_inc(dma_cwb, 16)
            if plan.has_bias:
                sync.dma_start(out=tmp3_sb[:], in_=clb_ext[:]).then_inc(dma_cwb, 16)
            # part 2: the big cluster
            sync.dma_start(out=w8_sb[:, :, WS:W],
                           in_=w8_ext[:, :, WS:W]).then_inc(dma_w2, 16)

            # misc for the epilogue
            sync.dma_start(out=oh_sb[:], in_=oh_ext[:]).then_inc(dma_misc, 16)
            sync.dma_start(out=bt_sb[:], in_=bt_ext[:]).then_inc(dma_misc, 16)
            # epilogue tiles, double-buffered, paced by the t-dot consumer
            for e in range(NB):
                if e >= 2:
                    sync.wait_ge(tdot_sem, 4 * (e - 1))
                sem_e = dma_ep0 if e % 2 == 0 else dma_ep1
                toff = (e % 2) * plan.hid
                sync.dma_start(out=xe_sb[:, toff:toff + plan.hid],
                               in_=xe_ext[e * PART:(e + 1) * PART, :]
                               ).then_inc(sem_e, 16)
                sync.dma_start(out=wt_sb[:, toff:toff + plan.hid],
                               in_=wt_ext[e * PART:(e + 1) * PART, :]
                               ).then_inc(sem_e, 16)
            # S partials out, AllReduce result back, final output
            sync.wait_ge(ve_sem, 1)
            sync.dma_start(out=ar_in[:], in_=s_sb[:]).then_inc(dma_out, 16)
            sync.wait_ge(cc_sem, 3)
            sync.dma_start(out=st_sb[:], in_=ar_out[:]).then_inc(dma_out, 16)
            sync.wait_ge(outv_sem, 1)
            sync.dma_start(out=out_ext[:], in_=fin_sb[:]).then_inc(dma_out, 16)

        @block.gpsimd
        def _(gpsimd):
            # tiny dummy collective issued immediately: pays the cold-start
            # and entry-barrier cost concurrently with the main compute, so
            # the real AllReduce at the end runs on a warm path
            gpsimd.dma_start(out=dm_in[:],
                             in_=bt_ext[:].rearrange("p e -> (p e)")[0:PART]
                             ).then_inc(gp_sem, 16)
            gpsimd.wait_ge(gp_sem, 16)
            gpsimd.collective_compute(
                "AllReduce",
                ALU.add,
                ins=[dm_in[:]],
                outs=[dm_out[:]],
                replica_groups=[list(range(N_CORES))],
            ).then_inc(cc_sem, 1)
            gpsimd.wait_ge(mm_sem, (G * 11) // 20)
            gpsimd.collective_compute(
                "AllReduce",
                ALU.add,
                ins=[dm_in[:]],
                outs=[dm_out[:]],
                replica_groups=[list(range(N_CORES))],
            ).then_inc(cc_sem, 1)
            gpsimd.wait_ge(dma_out, 16)
            gpsimd.collective_compute(
                "AllReduce",
                ALU.add,
                ins=[ar_in[:]],
                outs=[ar_out[:]],
                replica_groups=[list(range(N_CORES))],
            ).then_inc(cc_sem, 1)

        @block.tensor
        def _(tensor):
            tensor.wait_ge(dma_w0, 16 * n_p0)
            if plan.has_bias:
                tensor.wait_ge(veini_sem, 2)  # ones row ready
            for g, grp in enumerate(plan.groups):
                pb0 = pbase[grp["tidx"]]
                if g == plan.first_p1_group and n_p1 > 0:
                    tensor.wait_ge(dma_w1, 16 * n_p1)
                if g == plan.first_p2_group:
                    tensor.wait_ge(dma_w2, 32)
                if grp["prev_g"] is not None:
                    pa = plan.act_instrs[plan.act_of_group[grp["prev_g"]]]
                    tensor.wait_ge(act_sem if pa["eng"] == "act" else dvx_sem,
                                   pa["ord"] + 1)
                b = grp["b"]
                nunits = len(grp["units"])
                for ui, (po, wo, f) in enumerate(grp["units"]):
                    for j in range(plan.ndr):
                        if USE_SWI:
                            xoff = (b * plan.ndr + j) * 2 * PART
                            lhsT = x8_sb[:, xoff:xoff + 2 * PART]
                        else:
                            lhsT = x8_sb[:, 2 * j:2 * j + 2,
                                         b * PART:(b + 1) * PART]
                        mm = tensor.matmul(
                            ps[:, pb0 + po:pb0 + po + f],
                            lhsT=lhsT,
                            rhs=w8_sb[:, 2 * j:2 * j + 2, wo:wo + f],
                            start=(j == 0),
                            stop=(j == plan.ndr - 1 and not plan.has_bias),
                            perf_mode=DRSW if USE_SWI else DR)
                        if (j == plan.ndr - 1 and not plan.has_bias
                                and ui == nunits - 1):
                            mm.then_inc(mm_sem, 1)
                    if plan.has_bias:
                        mm = tensor.matmul(
                            ps[:, pb0 + po:pb0 + po + f],
                            lhsT=ones_sb[:],
                            rhs=brow_sb[0:1, wo:wo + f],
                            start=False, stop=True)
                        if ui == nunits - 1:
                            mm.then_inc(mm_sem, 1)

        @block.scalar
        def _(scalar):
            # x8 loads ride the ACT engine's parallel HWDGE ring
            def x8_dma(sem, tok_lo, tok_hi):
                if USE_SWI:
                    blo, bhi = tok_lo // PART, tok_hi // PART
                    clo, chi = blo * plan.ndr * 2 * PART, bhi * plan.ndr * 2 * PART
                    scalar.dma_start(
                        out=x8_sb[:, clo:chi],
                        in_=x8_ext[:, blo:bhi, :, :]).then_inc(sem, 16)
                else:
                    scalar.dma_start(out=x8_sb[:, :, tok_lo:tok_hi],
                                     in_=x8_ext[:, :, tok_lo:tok_hi]).then_inc(sem, 16)

            if TS0 > 0:
                x8_dma(dma_w0, 0, TS0)
            if TS > TS0:
                x8_dma(dma_w1, TS0, TS)
            x8_dma(dma_w2, TS, NTOK)
            scalar.wait_ge(veini_sem, 1)
            for a in plan.act_instrs:
                if a["eng"] != "act":
                    continue
                scalar.wait_ge(mm_sem, a["last_g"] + 1)
                o, sp = a["span_off"], a["span"]
                scalar.activation(
                    ps[:, o:o + sp],
                    ps[:, o:o + sp],
                    AF.Exp,
                    scale=plan.act_scale,
                    accum_out=sacc_sb[:, a["b"], a["slot"]:a["slot"] + 1],
                ).then_inc(act_sem, 1)
            # epilogue
            if plan.has_bias:
                scalar.wait_ge(ve2_sem, 2)
            else:
                scalar.wait_ge(tdot_sem, 4 * NB)
            scalar.activation(ecl_sb[:], cl_sb[:], AF.Exp).then_inc(fin_sem, 1)
            scalar.wait_ge(ve2_sem, 3 if plan.has_bias else 1)
            scalar.activation(lse3_sb[:], se3_sb[:], AF.Ln).then_inc(fin_sem, 1)
            scalar.wait_ge(dma_out, 32)
            scalar.activation(lns_sb[:], st_sb[:], AF.Ln).then_inc(fin_sem, 1)

        @block.vector
        def _(vector):
            vector.memset(sacc_sb[:], 0.0).then_inc(veini_sem, 1)
            if plan.has_bias:
                vector.memset(ones_sb[:], 1.0).then_inc(veini_sem, 1)
            vector.wait_ge(dma_cwb, 32 if plan.has_bias else 16)
            H = plan.hid
            for b in range(NB):
                # Schraudolph exp+sum for the previous block's offloaded tail
                # group goes first so its psum quarter frees as soon as the
                # matmuls finish
                for a in plan.dvx_of_block.get(b - 1, []):
                    o, sp = a["span_off"], a["span"]
                    vector.wait_ge(mm_sem, a["last_g"] + 1)
                    vector.tensor_scalar(
                        out=ps[:, o:o + sp].bitcast(I32),
                        in0=ps[:, o:o + sp],
                        scalar1=plan.dve_a,
                        scalar2=plan.dve_b,
                        op0=ALU.mult,
                        op1=ALU.add).then_inc(dvxp_sem, 1)
                    vector.wait_ge(dvxp_sem, a["ord"] + 1)
                    vector.reduce_sum(
                        sacc_sb[:, a["b"], a["slot"]:a["slot"] + 1],
                        ps[:, o:o + sp],
                        axis=mybir.AxisListType.X).then_inc(dvx_sem, 1)
                vector.wait_ge(dma_ep0 if b % 2 == 0 else dma_ep1,
                               32 * (b // 2 + 1))
                toff = (b % 2) * H
                # target-logit dot + 3 cluster-head dots, each with its own
                # scratch slot (WAW across tiles is ordered transitively via
                # the DMA pacing)
                po = (b % 2) * 4 * H
                vector.scalar_tensor_tensor(
                    out=prod_sb[:, po:po + H],
                    in0=xe_sb[:, toff:toff + H],
                    scalar=1.0,
                    in1=wt_sb[:, toff:toff + H],
                    op0=ALU.mult,
                    op1=ALU.mult,
                    accum_out=t_sb[:, b:b + 1],
                ).then_inc(tdot_sem, 1)
                for i in range(3):
                    vector.scalar_tensor_tensor(
                        out=prod_sb[:, po + (i + 1) * H:po + (i + 2) * H],
                        in0=xe_sb[:, toff:toff + H],
                        scalar=1.0,
                        in1=cwb_sb[:, i * H:(i + 1) * H],
                        op0=ALU.mult,
                        op1=ALU.mult,
                        accum_out=cl_sb[:, b, i:i + 1],
                    ).then_inc(tdot_sem, 1)
            for a in plan.dvx_of_block.get(NB - 1, []):
                o, sp = a["span_off"], a["span"]
                vector.wait_ge(mm_sem, a["last_g"] + 1)
                vector.tensor_scalar(
                    out=ps[:, o:o + sp].bitcast(I32),
                    in0=ps[:, o:o + sp],
                    scalar1=plan.dve_a,
                    scalar2=plan.dve_b,
                    op0=ALU.mult,
                    op1=ALU.add).then_inc(dvxp_sem, 1)
                vector.wait_ge(dvxp_sem, a["ord"] + 1)
                vector.reduce_sum(
                    sacc_sb[:, a["b"], a["slot"]:a["slot"] + 1],
                    ps[:, o:o + sp],
                    axis=mybir.AxisListType.X).then_inc(dvx_sem, 1)
            # ---- tail (serialized through vchain_sem for the race detector)
            vc = 0
            if plan.has_bias:
                # cl += cluster_b (clb staged in tmp3_sb)
                vector.wait_ge(tdot_sem, 4 * NB)
                vector.wait_ge(dma_cwb, 32)
                vector.tensor_tensor(cl_sb[:], cl_sb[:], tmp3_sb[:],
                                     ALU.add).then_inc(ve2_sem, 2)
            vector.wait_ge(act_sem, plan.n_act_eng)
            vector.wait_ge(dvx_sem, plan.n_dvx)
            vector.tensor_reduce(s_sb[:], sacc_sb[:], mybir.AxisListType.X,
                                 ALU.add).then_inc(ve_sem, 1)
            # cluster-head select (overlaps the AllReduce)
            vector.wait_ge(dma_misc, 16 * n_misc)
            if plan.has_bias:
                vector.wait_ge(ve2_sem, 2)
            else:
                vector.wait_ge(tdot_sem, 4 * NB)
            vector.tensor_tensor(tmp3_sb[:], cl_sb[:], oh_sb[:],
                                 ALU.mult).then_inc(vchain_sem, 1)
            vc += 1
            vector.wait_ge(vchain_sem, vc)
            vector.tensor_reduce(clsel_sb[:], tmp3_sb[:], mybir.AxisListType.X,
                                 ALU.add).then_inc(vchain_sem, 1)
            vc += 1
            vector.wait_ge(fin_sem, 1)
            vector.tensor_reduce(se3_sb[:], ecl_sb[:], mybir.AxisListType.X,
                                 ALU.add).then_inc(ve2_sem, 1)
            # pre-AR: w = lse3 - clsel - t - bt  (staged in lse3_sb)
            vector.wait_ge(fin_sem, 2)
            vector.scalar_tensor_tensor(out=lse3_sb[:], in0=lse3_sb[:], scalar=1.0,
                                        in1=clsel_sb[:], op0=ALU.mult,
                                        op1=ALU.subtract).then_inc(vchain_sem, 1)
            vc += 1
            vector.wait_ge(vchain_sem, vc)
            vector.scalar_tensor_tensor(out=lse3_sb[:], in0=lse3_sb[:], scalar=1.0,
                                        in1=t_sb[:], op0=ALU.mult,
                                        op1=ALU.subtract).then_inc(vchain_sem, 1)
            vc += 1
            vector.wait_ge(vchain_sem, vc)
            vector.scalar_tensor_tensor(out=lse3_sb[:], in0=lse3_sb[:], scalar=1.0,
                                        in1=bt_sb[:], op0=ALU.mult,
                                        op1=ALU.subtract).then_inc(vchain_sem, 1)
            vc += 1
            # post-AR: nll = lnS + w
            vector.wait_ge(fin_sem, 3)
            vector.wait_ge(vchain_sem, vc)
            vector.scalar_tensor_tensor(out=fin_sb[:], in0=lns_sb[:], scalar=1.0,
                                        in1=lse3_sb[:], op0=ALU.mult,
                                        op1=ALU.add).then_inc(outv_sem, 1)

    return nc


# ---------------------------------------------------------------------------
# host side


def _fp8(a, scale):
    return np.clip(np.asarray(a, np.float32) * scale, -240.0, 240.0).astype(
        ml_dtypes.float8_e4m3)


def _shard(x, y, cluster_w, cluster_b, logits_w, logits_b, cuts=CUTOFFS,
           group_cols=GROUP_COLS, mm_f=MM_F):
    x = np.asarray(x)
    y = np.asarray(y)
    cluster_w = np.asarray(cluster_w, dtype=np.float32)
    cluster_b = np.asarray(cluster_b, dtype=np.float32)
    logits_w = np.asarray(logits_w, dtype=np.float32)
    logits_b = np.asarray(logits_b, dtype=np.float32)

    xf = np.ascontiguousarray(x[:, :-1]).reshape(-1, x.shape[-1]).astype(np.float32)
    yf = y.reshape(-1).astype(np.int64)
    n = xf.shape[0]
    hid = xf.shape[1]
    ncl = len(cuts) - 1
    hg = hid // PART

    cid = np.zeros(n, dtype=np.int64)
    for i in range(1, ncl):
        cid += yf >= cuts[i]

    order = np.argsort(cid, kind="stable")
    counts = np.bincount(cid, minlength=ncl)
    bpc = [int(-(-c // PART)) for c in counts]
    nb = sum(bpc)
    ntok = nb * PART

    dev_orig = np.full(ntok, -1, dtype=np.int64)
    y_dev = np.empty(ntok, dtype=np.int64)
    cid_dev = np.empty(ntok, dtype=np.int64)
    pos = 0
    spos = 0
    for ci in range(ncl):
        cnt = int(counts[ci])
        seg = order[spos:spos + cnt]
        dev_orig[pos:pos + cnt] = seg
        y_dev[pos:pos + cnt] = yf[seg]
        y_dev[pos + cnt:pos + bpc[ci] * PART] = cuts[ci]
        cid_dev[pos:pos + bpc[ci] * PART] = ci
        pos += bpc[ci] * PART
        spos += cnt

    xf_dev = np.zeros((ntok, hid), dtype=np.float32)
    real = dev_orig >= 0
    xf_dev[real] = xf[dev_orig[real]]

    bf = ml_dtypes.bfloat16
    # fp8 DoubleRow layout: [p, g, tok] with contraction k = g*128 + p
    x8g = _fp8(xf_dev.T, SCALE_X).reshape(hg, PART, ntok)
    if USE_SWI:
        # DoubleRowSwInterleave stationary layout: per (block b, pair j):
        # sw[p, 2k+i] = x[(2j+i)*128+p, b*128 + (127-k)]
        ndr = hg // 2
        a = x8g.reshape(ndr, 2, PART, nb, PART)        # [j, i, p, b, tok]
        a = a[:, :, :, :, ::-1]                        # reverse tokens
        # -> [p, b, j, tok, i]
        a = a.transpose(2, 3, 0, 4, 1)
        x8 = np.ascontiguousarray(a.reshape(PART, nb, ndr, 2 * PART))
    else:
        x8 = np.ascontiguousarray(x8g.transpose(1, 0, 2))
    xe = np.ascontiguousarray(xf_dev).astype(bf)             # [ntok, H]
    wt = np.ascontiguousarray(logits_w.T[y_dev]).astype(bf)  # [ntok, H]

    bt = logits_b[0, y_dev].astype(np.float32).reshape(nb, PART).T.copy()
    oh = np.zeros((ntok, 3), dtype=np.float32)
    oh[np.arange(ntok), cid_dev] = 1.0
    oh = np.ascontiguousarray(oh.reshape(nb, PART, 3).transpose(1, 0, 2))

    has_bias = bool(logits_b.any() or cluster_b.any())
    widths = []
    for ci in range(ncl):
        v = cuts[ci + 1] - cuts[ci]
        assert v % N_CORES == 0
        widths.append(v // N_CORES)

    cwb = np.ascontiguousarray(np.broadcast_to(
        cluster_w.T.reshape(1, 3 * hid), (PART, 3 * hid))).astype(bf)
    clb = np.ascontiguousarray(np.broadcast_to(
        cluster_b.reshape(1, 1, 3), (PART, nb, 3))).astype(np.float32)

    w_cores = []
    brow_cores = []
    bscale = SCALE_W * SCALE_X
    for c in range(N_CORES):
        parts = []
        bparts = []
        for ci in range(ncl):
            lo = cuts[ci] + c * widths[ci]
            parts.append(logits_w[:, lo:lo + widths[ci]])
            bparts.append(logits_b[:, lo:lo + widths[ci]] * bscale)
        wc = np.concatenate(parts, 1)                       # [hid, W]
        w8 = np.ascontiguousarray(
            _fp8(wc, SCALE_W).reshape(hg, PART, -1).transpose(1, 0, 2))
        w_cores.append(w8)
        brow_cores.append(np.ascontiguousarray(np.concatenate(bparts, 1)).astype(bf))

    plan = Plan(bpc, widths, has_bias, group_cols=group_cols, hid=hid, mm_f=mm_f)

    in_maps = []
    for c in range(N_CORES):
        m = dict(x8=x8, w8=w_cores[c], xe=xe, wt=wt, oh=oh, bt=bt, cwb=cwb)
        if has_bias:
            m["brow"] = brow_cores[c]
            m["clb"] = clb
        in_maps.append(m)

    meta = dict(dev_orig=dev_orig, n=n, nb=nb)
    return plan, in_maps, meta


def _unshard(out, meta):
    nll_dev = np.ascontiguousarray(np.asarray(out, dtype=np.float32).T).reshape(-1)
    res = np.zeros(meta["n"], dtype=np.float32)
    real = meta["dev_orig"] >= 0
    res[meta["dev_orig"][real]] = nll_dev[real]
    return res


def kernel(x, y, cluster_w, cluster_b, logits_w, logits_b):
    plan, in_maps, meta = _shard(x, y, cluster_w, cluster_b, logits_w, logits_b)
    nc = build_graph(plan)
    res = run_bass_kernel_spmd(nc, in_maps, list(range(N_CORES)))
    return _unshard(res.results[0]["out"], meta)


# revision 44
# speedup vs baseline: 1.4241x; 1.0089x over previous
"""Adaptive-softmax NLL loss on 8 Trainium2 NeuronCores.

Algorithm (cluster-sparse): per token only its own cluster's log-softmax
matters, so
    nll[t] = -( cl[t, c(t)] - LSE(cl[t,:]) + logit[t, y_t] - ln S[t] )
with  S[t] = sum_{j in cluster(y_t)} exp(x_t . W[:,j] + b_j).

Sharding: tokens are cluster-sorted into 128-row blocks; each cluster's
vocab range is split evenly across the 8 cores (tensor parallel over
vocab).  Every core computes partial S for all tokens over its vocab
slice (fp8 DoubleRow matmul -> ScalarE exp with free-axis accumulate),
the partials are combined with a single small AllReduce, and each core
finishes the per-token epilogue locally.  The target logit is computed
from the host-gathered columns W[:, y] as an elementwise bf16 dot on
VectorE.  The 3-column cluster head rides along as 3 extra weight
columns.  fp8 inputs are pre-scaled by powers of two on the host; the
exp's built-in scale multiplier unwinds the scaling for free.
"""

import numpy as np
import ml_dtypes
from contextlib import ExitStack

import concourse.bass as bass
import concourse.mybir as mybir
from concourse.bass_utils import run_bass_kernel_spmd

F32 = mybir.dt.float32
BF16 = mybir.dt.bfloat16
FP8 = mybir.dt.float8e4
AF = mybir.ActivationFunctionType
ALU = mybir.AluOpType
DR = mybir.MatmulPerfMode.DoubleRow
DRSW = mybir.MatmulPerfMode.DoubleRowSwInterleave
USE_SWI = False

N_CORES = 8
PART = 128
CUTOFFS = [0, 2000, 10000, 50000]
HID = 512

GROUP_COLS = 1024   # retained for the small-scale sim configs
TCAPS = [1024, 1024, 1024, 1024]   # psum tensor widths (2 banks each)
MM_F = 512          # max matmul free size (one psum bank)
SCALE_W = 2048.0    # fp8 pre-scale for weights (power of 2)
SCALE_X = 32.0      # fp8 pre-scale for activations (power of 2)

DISABLE = set()     # bisection hooks


# ---------------------------------------------------------------------------
# planning


class Plan:
    """Static schedule shared by the host sharding code and the builder."""

    def __init__(self, blocks_per_cluster, widths, has_bias, group_cols=GROUP_COLS,
                 hid=HID, mm_f=MM_F):
        assert hid % 256 == 0
        self.hg = hid // PART          # 128-row h-groups (4)
        self.ndr = hid // 256          # DoubleRow matmuls per unit (2)
        self.hid = hid
        self.has_bias = has_bias
        self.group_cols = group_cols
        self.mm_f = mm_f
        self.widths = widths                      # per-core cols per cluster
        self.bpc = blocks_per_cluster             # blocks per cluster
        self.nb = sum(blocks_per_cluster)
        self.ncl = len(widths)
        self.act_scale = 1.0 / (SCALE_W * SCALE_X)

        # per-core w column layout: [c0 | c1 | ... ] (cluster head is
        # computed on VectorE from bf16 inputs instead)
        self.w_off = []
        off = 0
        for wd in widths:
            self.w_off.append(off)
            off += wd
        self.wcols = off

        # head-split: part0 = first cluster only; part1 = all clusters
        # except the last; part2 = the big last cluster
        self.wsplit0 = sum(widths[:-1])
        self.tsplit0 = PART * sum(blocks_per_cluster[:-1])
        self.wsplit = self.w_off[-1]
        self.tsplit = PART * sum(blocks_per_cluster[:-1])

        # blocks: cluster index per block
        self.block_cluster = []
        for ci, nblk in enumerate(blocks_per_cluster):
            self.block_cluster += [ci] * nblk

        # groups: the unit of PSUM rotation.  Asymmetric psum tensors,
        # assigned round-robin (LRU); each group is one ACT exp+accum.
        if group_cols == GROUP_COLS:
            self.tcaps = list(TCAPS)
        else:                      # small-scale sim: 4 tensors of group_cols
            self.tcaps = [group_cols] * 4
        self.groups = []   # dicts: b, gi, tidx, prev_g, units[(po,wo,F)], span
        lru = list(range(len(self.tcaps)))
        last_on = [None] * len(self.tcaps)
        for b, ci in enumerate(self.block_cluster):
            V = widths[ci]
            wo0 = self.w_off[ci]
            col = 0
            gi = 0
            while col < V:
                t = lru.pop(0)
                lru.append(t)
                gsz = min(self.tcaps[t], V - col)
                units = []
                po = 0
                rem = gsz
                while rem > 0:
                    f = min(self.mm_f, rem)
                    units.append((po, wo0 + col + po, f))
                    po += f
                    rem -= f
                g = len(self.groups)
                self.groups.append(dict(b=b, gi=gi, tidx=t, prev_g=last_on[t],
                                        units=units, span=gsz))
                last_on[t] = g
                col += gsz
                gi += 1
        self.ngroups = len(self.groups)

        # fuse ACT over pairs of groups in adjacent psum quarters (the psum
        # is one contiguous tensor; consecutive tidx => contiguous columns)
        self.act_instrs = []    # dicts: b, span_off, span, slot, last_g
        self.act_of_group = [None] * self.ngroups
        caps = self.tcaps
        g = 0
        while g < self.ngroups:
            grp = self.groups[g]
            b = grp["b"]
            fuse = False
            if g + 1 < self.ngroups:
                nxt = self.groups[g + 1]
                if (nxt["b"] == b and nxt["tidx"] == grp["tidx"] + 1
                        and grp["span"] == caps[grp["tidx"]]):
                    fuse = True
            off = sum(caps[:grp["tidx"]])
            if fuse:
                span = grp["span"] + self.groups[g + 1]["span"]
                idx = len(self.act_instrs)
                self.act_of_group[g] = idx
                self.act_of_group[g + 1] = idx
                slot = len([a for a in self.act_instrs if a["b"] == b])
                self.act_instrs.append(dict(b=b, span_off=off, span=span,
                                            slot=slot, last_g=g + 1))
                g += 2
            else:
                idx = len(self.act_instrs)
                self.act_of_group[g] = idx
                slot = len([a for a in self.act_instrs if a["b"] == b])
                self.act_instrs.append(dict(b=b, span_off=off, span=grp["span"],
                                            slot=slot, last_g=g))
                g += 1
        self.n_act = len(self.act_instrs)
        self.max_gpb = max(a["slot"] for a in self.act_instrs) + 1

        # first group needing part1 (middle clusters) / part2 (last cluster)
        self.first_p1_group = None
        self.first_p2_group = None
        for g, grp in enumerate(self.groups):
            ci = self.block_cluster[grp["b"]]
            if ci not in (0, self.ncl - 1) and self.first_p1_group is None:
                self.first_p1_group = g
            if ci == self.ncl - 1 and self.first_p2_group is None:
                self.first_p2_group = g
                break


def build_graph(plan: Plan):
    nc = bass.Bass()
    HG, NB, G = plan.hg, plan.nb, plan.ngroups
    NTOK = NB * PART
    W = plan.wcols
    WS, TS = plan.wsplit, plan.tsplit

    if USE_SWI:
        x8_ext = nc.declare_dram_parameter("x8", [PART, NB, plan.ndr, 2 * PART],
                                           FP8, isOutput=False)
    else:
        x8_ext = nc.declare_dram_parameter("x8", [PART, HG, NTOK], FP8,
                                           isOutput=False)
    w8_ext = nc.declare_dram_parameter("w8", [PART, HG, W], FP8, isOutput=False)
    xe_ext = nc.declare_dram_parameter("xe", [NTOK, plan.hid], BF16, isOutput=False)
    wt_ext = nc.declare_dram_parameter("wt", [NTOK, plan.hid], BF16, isOutput=False)
    cwb_ext = nc.declare_dram_parameter("cwb", [PART, 3 * plan.hid], BF16,
                                        isOutput=False)
    oh_ext = nc.declare_dram_parameter("oh", [PART, NB, 3], F32, isOutput=False)
    bt_ext = nc.declare_dram_parameter("bt", [PART, NB], F32, isOutput=False)
    if plan.has_bias:
        brow_ext = nc.declare_dram_parameter("brow", [1, W], BF16, isOutput=False)
        clb_ext = nc.declare_dram_parameter("clb", [PART, NB, 3], F32,
                                            isOutput=False)
    out_ext = nc.declare_dram_parameter("out", [PART, NB], F32, isOutput=True)

    ar_in = nc.dram_tensor("ar_in", [PART, NB], F32)
    ar_out = nc.dram_tensor("ar_out", [PART, NB], F32, addr_space="Shared")
    dm_in = nc.dram_tensor("dm_in", [PART], F32)
    dm_out = nc.dram_tensor("dm_out", [PART], F32, addr_space="Shared")

    n_p0 = 1 + (1 if plan.tsplit0 > 0 else 0) + (1 if plan.has_bias else 0)
    n_p1 = ((1 if plan.wsplit > plan.wsplit0 else 0)
            + (1 if plan.tsplit > plan.tsplit0 else 0))
    n_misc = 2                               # oh, bt

    with ExitStack() as ctx:
        w8_sb = ctx.enter_context(nc.sbuf_tensor([PART, HG, W], FP8))
        if USE_SWI:
            x8_sb = ctx.enter_context(
                nc.sbuf_tensor([PART, NB * plan.ndr * 2 * PART], FP8))
        else:
            x8_sb = ctx.enter_context(nc.sbuf_tensor([PART, HG, NTOK], FP8))
        xe_sb = ctx.enter_context(nc.sbuf_tensor([PART, 2 * plan.hid], BF16))
        wt_sb = ctx.enter_context(nc.sbuf_tensor([PART, 2 * plan.hid], BF16))
        sacc_sb = ctx.enter_context(nc.sbuf_tensor([PART, NB, plan.max_gpb], F32))
        cl_sb = ctx.enter_context(nc.sbuf_tensor([PART, NB, 3], F32))
        ecl_sb = ctx.enter_context(nc.sbuf_tensor([PART, NB, 3], F32))
        oh_sb = ctx.enter_context(nc.sbuf_tensor([PART, NB, 3], F32))
        tmp3_sb = ctx.enter_context(nc.sbuf_tensor([PART, NB, 3], F32))
        prod_sb = ctx.enter_context(nc.sbuf_tensor([PART, 8 * plan.hid], F32))
        cwb_sb = ctx.enter_context(nc.sbuf_tensor([PART, 3 * plan.hid], BF16))
        t_sb = ctx.enter_context(nc.sbuf_tensor([PART, NB], F32))
        bt_sb = ctx.enter_context(nc.sbuf_tensor([PART, NB], F32))
        s_sb = ctx.enter_context(nc.sbuf_tensor([PART, NB], F32))
        st_sb = ctx.enter_context(nc.sbuf_tensor([PART, NB], F32))
        lns_sb = ctx.enter_context(nc.sbuf_tensor([PART, NB], F32))
        se3_sb = ctx.enter_context(nc.sbuf_tensor([PART, NB], F32))
        lse3_sb = ctx.enter_context(nc.sbuf_tensor([PART, NB], F32))
        clsel_sb = ctx.enter_context(nc.sbuf_tensor([PART, NB], F32))
        fin_sb = ctx.enter_context(nc.sbuf_tensor([PART, NB], F32))
        ones_sb = ctx.enter_context(nc.sbuf_tensor([1, PART], BF16))
        brow_sb = ctx.enter_context(nc.sbuf_tensor([1, W], BF16))
        ps = ctx.enter_context(nc.psum_tensor("ps",
                                              [PART, sum(plan.tcaps)], F32))
        pbase = [sum(plan.tcaps[:i]) for i in range(len(plan.tcaps))]
        dma_w0 = ctx.enter_context(nc.semaphore("dma_w0"))
        dma_w1 = ctx.enter_context(nc.semaphore("dma_w1"))
        dma_w2 = ctx.enter_context(nc.semaphore("dma_w2"))
        dma_misc = ctx.enter_context(nc.semaphore("dma_misc"))
        dma_ep0 = ctx.enter_context(nc.semaphore("dma_ep0"))
        dma_ep1 = ctx.enter_context(nc.semaphore("dma_ep1"))
        dma_out = ctx.enter_context(nc.semaphore("dma_out"))
        mm_sem = ctx.enter_context(nc.semaphore("mm_sem"))
        act_sem = ctx.enter_context(nc.semaphore("act_sem"))
        dma_cwb = ctx.enter_context(nc.semaphore("dma_cwb"))
        tdot_sem = ctx.enter_context(nc.semaphore("tdot_sem"))
        veini_sem = ctx.enter_context(nc.semaphore("veini_sem"))
        ve_sem = ctx.enter_context(nc.semaphore("ve_sem"))
        ve2_sem = ctx.enter_context(nc.semaphore("ve2_sem"))
        cc_sem = ctx.enter_context(nc.semaphore("cc_sem"))
        fin_sem = ctx.enter_context(nc.semaphore("fin_sem"))
        outv_sem = ctx.enter_context(nc.semaphore("outv_sem"))
        vchain_sem = ctx.enter_context(nc.semaphore("vchain_sem"))
        gp_sem = ctx.enter_context(nc.semaphore("gp_sem"))
        block = ctx.enter_context(nc.Block())

        WS0, TS0 = plan.wsplit0, plan.tsplit0

        @block.sync
        def _(sync):
            def x8_dma(sem, tok_lo, tok_hi):
                if USE_SWI:
                    blo, bhi = tok_lo // PART, tok_hi // PART
                    clo, chi = blo * plan.ndr * 2 * PART, bhi * plan.ndr * 2 * PART
                    sync.dma_start(
                        out=x8_sb[:, clo:chi],
                        in_=x8_ext[:, blo:bhi, :, :]).then_inc(sem, 16)
                else:
                    sync.dma_start(out=x8_sb[:, :, tok_lo:tok_hi],
                                   in_=x8_ext[:, :, tok_lo:tok_hi]).then_inc(sem, 16)

            # part 0: just the first cluster's slice, to start PE asap
            sync.dma_start(out=w8_sb[:, :, 0:WS0],
                           in_=w8_ext[:, :, 0:WS0]).then_inc(dma_w0, 16)
            if TS0 > 0:
                x8_dma(dma_w0, 0, TS0)
            # part 1: remaining small clusters (empty when no middle part)
            if WS > WS0:
                sync.dma_start(out=w8_sb[:, :, WS0:WS],
                               in_=w8_ext[:, :, WS0:WS]).then_inc(dma_w1, 16)
            if TS > TS0:
                x8_dma(dma_w1, TS0, TS)
            if plan.has_bias:
                sync.dma_start(out=brow_sb[:], in_=brow_ext[:]).then_inc(dma_w0, 16)
            sync.dma_start(out=cwb_sb[:], in_=cwb_ext[:]).then_inc(dma_cwb, 16)
            if plan.has_bias:
                sync.dma_start(out=tmp3_sb[:], in_=clb_ext[:]).then_inc(dma_cwb, 16)
            # part 2: the big cluster
            sync.dma_start(out=w8_sb[:, :, WS:W],
                           in_=w8_ext[:, :, WS:W]).then_inc(dma_w2, 16)
            x8_dma(dma_w2, TS, NTOK)
            # misc for the epilogue
            sync.dma_start(out=oh_sb[:], in_=oh_ext[:]).then_inc(dma_misc, 16)
            sync.dma_start(out=bt_sb[:], in_=bt_ext[:]).then_inc(dma_misc, 16)
            # epilogue tiles, double-buffered, paced by the t-dot consumer
            for e in range(NB):
                if e >= 2:
                    sync.wait_ge(tdot_sem, 4 * (e - 1))
                sem_e = dma_ep0 if e % 2 == 0 else dma_ep1
                toff = (e % 2) * plan.hid
                sync.dma_start(out=xe_sb[:, toff:toff + plan.hid],
                               in_=xe_ext[e * PART:(e + 1) * PART, :]
                               ).then_inc(sem_e, 16)
                sync.dma_start(out=wt_sb[:, toff:toff + plan.hid],
                               in_=wt_ext[e * PART:(e + 1) * PART, :]
                               ).then_inc(sem_e, 16)
            # S partials out, AllReduce result back, final output
            sync.wait_ge(ve_sem, 1)
            sync.dma_start(out=ar_in[:], in_=s_sb[:]).then_inc(dma_out, 16)
            sync.wait_ge(cc_sem, 3)
            sync.dma_start(out=st_sb[:], in_=ar_out[:]).then_inc(dma_out, 16)
            sync.wait_ge(outv_sem, 1)
            sync.dma_start(out=out_ext[:], in_=fin_sb[:]).then_inc(dma_out, 16)

        @block.gpsimd
        def _(gpsimd):
            # tiny dummy collective issued immediately: pays the cold-start
            # and entry-barrier cost concurrently with the main compute, so
            # the real AllReduce at the end runs on a warm path
            gpsimd.dma_start(out=dm_in[:],
                             in_=bt_ext[:].rearrange("p e -> (p e)")[0:PART]
                             ).then_inc(gp_sem, 16)
            gpsimd.wait_ge(gp_sem, 16)
            gpsimd.collective_compute(
                "AllReduce",
                ALU.add,
                ins=[dm_in[:]],
                outs=[dm_out[:]],
                replica_groups=[list(range(N_CORES))],
            ).then_inc(cc_sem, 1)
            gpsimd.wait_ge(mm_sem, (G * 11) // 20)
            gpsimd.collective_compute(
                "AllReduce",
                ALU.add,
                ins=[dm_in[:]],
                outs=[dm_out[:]],
                replica_groups=[list(range(N_CORES))],
            ).then_inc(cc_sem, 1)
            gpsimd.wait_ge(dma_out, 16)
            gpsimd.collective_compute(
                "AllReduce",
                ALU.add,
                ins=[ar_in[:]],
                outs=[ar_out[:]],
                replica_groups=[list(range(N_CORES))],
            ).then_inc(cc_sem, 1)

        @block.tensor
        def _(tensor):
            tensor.wait_ge(dma_w0, 16 * n_p0)
            if plan.has_bias:
                tensor.wait_ge(veini_sem, 2)  # ones row ready
            for g, grp in enumerate(plan.groups):
                pb0 = pbase[grp["tidx"]]
                if g == plan.first_p1_group and n_p1 > 0:
                    tensor.wait_ge(dma_w1, 16 * n_p1)
                if g == plan.first_p2_group:
                    tensor.wait_ge(dma_w2, 32)
                if grp["prev_g"] is not None:
                    tensor.wait_ge(act_sem,
                                   plan.act_of_group[grp["prev_g"]] + 1)
                b = grp["b"]
                nunits = len(grp["units"])
                for ui, (po, wo, f) in enumerate(grp["units"]):
                    for j in range(plan.ndr):
                        if USE_SWI:
                            xoff = (b * plan.ndr + j) * 2 * PART
                            lhsT = x8_sb[:, xoff:xoff + 2 * PART]
                        else:
                            lhsT = x8_sb[:, 2 * j:2 * j + 2,
                                         b * PART:(b + 1) * PART]
                        mm = tensor.matmul(
                            ps[:, pb0 + po:pb0 + po + f],
                            lhsT=lhsT,
                            rhs=w8_sb[:, 2 * j:2 * j + 2, wo:wo + f],
                            start=(j == 0),
                            stop=(j == plan.ndr - 1 and not plan.has_bias),
                            perf_mode=DRSW if USE_SWI else DR)
                        if (j == plan.ndr - 1 and not plan.has_bias
                                and ui == nunits - 1):
                            mm.then_inc(mm_sem, 1)
                    if plan.has_bias:
                        mm = tensor.matmul(
                            ps[:, pb0 + po:pb0 + po + f],
                            lhsT=ones_sb[:],
                            rhs=brow_sb[0:1, wo:wo + f],
                            start=False, stop=True)
                        if ui == nunits - 1:
                            mm.then_inc(mm_sem, 1)

        @block.scalar
        def _(scalar):
            scalar.wait_ge(veini_sem, 1)
            for ai, a in enumerate(plan.act_instrs):
                scalar.wait_ge(mm_sem, a["last_g"] + 1)
                o, sp = a["span_off"], a["span"]
                scalar.activation(
                    ps[:, o:o + sp],
                    ps[:, o:o + sp],
                    AF.Exp,
                    scale=plan.act_scale,
                    accum_out=sacc_sb[:, a["b"], a["slot"]:a["slot"] + 1],
                ).then_inc(act_sem, 1)
            # epilogue
            if plan.has_bias:
                scalar.wait_ge(ve2_sem, 2)
            else:
                scalar.wait_ge(tdot_sem, 4 * NB)
            scalar.activation(ecl_sb[:], cl_sb[:], AF.Exp).then_inc(fin_sem, 1)
            scalar.wait_ge(ve2_sem, 3 if plan.has_bias else 1)
            scalar.activation(lse3_sb[:], se3_sb[:], AF.Ln).then_inc(fin_sem, 1)
            scalar.wait_ge(dma_out, 32)
            scalar.activation(lns_sb[:], st_sb[:], AF.Ln).then_inc(fin_sem, 1)

        @block.vector
        def _(vector):
            vector.memset(sacc_sb[:], 0.0).then_inc(veini_sem, 1)
            if plan.has_bias:
                vector.memset(ones_sb[:], 1.0).then_inc(veini_sem, 1)
            vector.wait_ge(dma_cwb, 32 if plan.has_bias else 16)
            H = plan.hid
            for b in range(NB):
                vector.wait_ge(dma_ep0 if b % 2 == 0 else dma_ep1,
                               32 * (b // 2 + 1))
                toff = (b % 2) * H
                # target-logit dot + 3 cluster-head dots, each with its own
                # scratch slot (WAW across tiles is ordered transitively via
                # the DMA pacing)
                po = (b % 2) * 4 * H
                vector.scalar_tensor_tensor(
                    out=prod_sb[:, po:po + H],
                    in0=xe_sb[:, toff:toff + H],
                    scalar=1.0,
                    in1=wt_sb[:, toff:toff + H],
                    op0=ALU.mult,
                    op1=ALU.mult,
                    accum_out=t_sb[:, b:b + 1],
                ).then_inc(tdot_sem, 1)
                for i in range(3):
                    vector.scalar_tensor_tensor(
                        out=prod_sb[:, po + (i + 1) * H:po + (i + 2) * H],
                        in0=xe_sb[:, toff:toff + H],
                        scalar=1.0,
                        in1=cwb_sb[:, i * H:(i + 1) * H],
                        op0=ALU.mult,
                        op1=ALU.mult,
                        accum_out=cl_sb[:, b, i:i + 1],
                    ).then_inc(tdot_sem, 1)
            # ---- tail (serialized through vchain_sem for the race detector)
            vc = 0
            if plan.has_bias:
                # cl += cluster_b (clb staged in tmp3_sb)
                vector.wait_ge(tdot_sem, 4 * NB)
                vector.wait_ge(dma_cwb, 32)
                vector.tensor_tensor(cl_sb[:], cl_sb[:], tmp3_sb[:],
                                     ALU.add).then_inc(ve2_sem, 2)
            vector.wait_ge(act_sem, plan.n_act)
            vector.tensor_reduce(s_sb[:], sacc_sb[:], mybir.AxisListType.X,
                                 ALU.add).then_inc(ve_sem, 1)
            # cluster-head select (overlaps the AllReduce)
            vector.wait_ge(dma_misc, 16 * n_misc)
            if plan.has_bias:
                vector.wait_ge(ve2_sem, 2)
            else:
                vector.wait_ge(tdot_sem, 4 * NB)
            vector.tensor_tensor(tmp3_sb[:], cl_sb[:], oh_sb[:],
                                 ALU.mult).then_inc(vchain_sem, 1)
            vc += 1
            vector.wait_ge(vchain_sem, vc)
            vector.tensor_reduce(clsel_sb[:], tmp3_sb[:], mybir.AxisListType.X,
                                 ALU.add).then_inc(vchain_sem, 1)
            vc += 1
            vector.wait_ge(fin_sem, 1)
            vector.tensor_reduce(se3_sb[:], ecl_sb[:], mybir.AxisListType.X,
                                 ALU.add).then_inc(ve2_sem, 1)
            # pre-AR: w = lse3 - clsel - t - bt  (staged in lse3_sb)
            vector.wait_ge(fin_sem, 2)
            vector.scalar_tensor_tensor(out=lse3_sb[:], in0=lse3_sb[:], scalar=1.0,
                                        in1=clsel_sb[:], op0=ALU.mult,
                                        op1=ALU.subtract).then_inc(vchain_sem, 1)
            vc += 1
            vector.wait_ge(vchain_sem, vc)
            vector.scalar_tensor_tensor(out=lse3_sb[:], in0=lse3_sb[:], scalar=1.0,
                                        in1=t_sb[:], op0=ALU.mult,
                                        op1=ALU.subtract).then_inc(vchain_sem, 1)
            vc += 1
            vector.wait_ge(vchain_sem, vc)
            vector.scalar_tensor_tensor(out=lse3_sb[:], in0=lse3_sb[:], scalar=1.0,
                                        in1=bt_sb[:], op0=ALU.mult,
                                        op1=ALU.subtract).then_inc(vchain_sem, 1)
            vc += 1
            # post-AR: nll = lnS + w
            vector.wait_ge(fin_sem, 3)
            vector.wait_ge(vchain_sem, vc)
            vector.scalar_tensor_tensor(out=fin_sb[:], in0=lns_sb[:], scalar=1.0,
                                        in1=lse3_sb[:], op0=ALU.mult,
                                        op1=ALU.add).then_inc(outv_sem, 1)

    return nc


# ---------------------------------------------------------------------------
# host side


def _fp8(a, scale):
    return np.clip(np.asarray(a, np.float32) * scale, -240.0, 240.0).astype(
        ml_dtypes.float8_e4m3)


def _shard(x, y, cluster_w, cluster_b, logits_w, logits_b, cuts=CUTOFFS,
           group_cols=GROUP_COLS, mm_f=MM_F):
    x = np.asarray(x)
    y = np.asarray(y)
    cluster_w = np.asarray(cluster_w, dtype=np.float32)
    cluster_b = np.asarray(cluster_b, dtype=np.float32)
    logits_w = np.asarray(logits_w, dtype=np.float32)
    logits_b = np.asarray(logits_b, dtype=np.float32)

    xf = np.ascontiguousarray(x[:, :-1]).reshape(-1, x.shape[-1]).astype(np.float32)
    yf = y.reshape(-1).astype(np.int64)
    n = xf.shape[0]
    hid = xf.shape[1]
    ncl = len(cuts) - 1
    hg = hid // PART

    cid = np.zeros(n, dtype=np.int64)
    for i in range(1, ncl):
        cid += yf >= cuts[i]

    order = np.argsort(cid, kind="stable")
    counts = np.bincount(cid, minlength=ncl)
    bpc = [int(-(-c // PART)) for c in counts]
    nb = sum(bpc)
    ntok = nb * PART

    dev_orig = np.full(ntok, -1, dtype=np.int64)
    y_dev = np.empty(ntok, dtype=np.int64)
    cid_dev = np.empty(ntok, dtype=np.int64)
    pos = 0
    spos = 0
    for ci in range(ncl):
        cnt = int(counts[ci])
        seg = order[spos:spos + cnt]
        dev_orig[pos:pos + cnt] = seg
        y_dev[pos:pos + cnt] = yf[seg]
        y_dev[pos + cnt:pos + bpc[ci] * PART] = cuts[ci]
        cid_dev[pos:pos + bpc[ci] * PART] = ci
        pos += bpc[ci] * PART
        spos += cnt

    xf_dev = np.zeros((ntok, hid), dtype=np.float32)
    real = dev_orig >= 0
    xf_dev[real] = xf[dev_orig[real]]

    bf = ml_dtypes.bfloat16
    # fp8 DoubleRow layout: [p, g, tok] with contraction k = g*128 + p
    x8g = _fp8(xf_dev.T, SCALE_X).reshape(hg, PART, ntok)
    if USE_SWI:
        # DoubleRowSwInterleave stationary layout: per (block b, pair j):
        # sw[p, 2k+i] = x[(2j+i)*128+p, b*128 + (127-k)]
        ndr = hg // 2
        a = x8g.reshape(ndr, 2, PART, nb, PART)        # [j, i, p, b, tok]
        a = a[:, :, :, :, ::-1]                        # reverse tokens
        # -> [p, b, j, tok, i]
        a = a.transpose(2, 3, 0, 4, 1)
        x8 = np.ascontiguousarray(a.reshape(PART, nb, ndr, 2 * PART))
    else:
        x8 = np.ascontiguousarray(x8g.transpose(1, 0, 2))
    xe = np.ascontiguousarray(xf_dev).astype(bf)             # [ntok, H]
    wt = np.ascontiguousarray(logits_w.T[y_dev]).astype(bf)  # [ntok, H]

    bt = logits_b[0, y_dev].astype(np.float32).reshape(nb, PART).T.copy()
    oh = np.zeros((ntok, 3), dtype=np.float32)
    oh[np.arange(ntok), cid_dev] = 1.0
    oh = np.ascontiguousarray(oh.reshape(nb, PART, 3).transpose(1, 0, 2))

    has_bias = bool(logits_b.any() or cluster_b.any())
    widths = []
    for ci in range(ncl):
        v = cuts[ci + 1] - cuts[ci]
        assert v % N_CORES == 0
        widths.append(v // N_CORES)

    cwb = np.ascontiguousarray(np.broadcast_to(
        cluster_w.T.reshape(1, 3 * hid), (PART, 3 * hid))).astype(bf)
    clb = np.ascontiguousarray(np.broadcast_to(
        cluster_b.reshape(1, 1, 3), (PART, nb, 3))).astype(np.float32)

    w_cores = []
    brow_cores = []
    bscale = SCALE_W * SCALE_X
    for c in range(N_CORES):
        parts = []
        bparts = []
        for ci in range(ncl):
            lo = cuts[ci] + c * widths[ci]
            parts.append(logits_w[:, lo:lo + widths[ci]])
            bparts.append(logits_b[:, lo:lo + widths[ci]] * bscale)
        wc = np.concatenate(parts, 1)                       # [hid, W]
        w8 = np.ascontiguousarray(
            _fp8(wc, SCALE_W).reshape(hg, PART, -1).transpose(1, 0, 2))
        w_cores.append(w8)
        brow_cores.append(np.ascontiguousarray(np.concatenate(bparts, 1)).astype(bf))

    plan = Plan(bpc, widths, has_bias, group_cols=group_cols, hid=hid, mm_f=mm_f)

    in_maps = []
    for c in range(N_CORES):
        m = dict(x8=x8, w8=w_cores[c], xe=xe, wt=wt, oh=oh, bt=bt, cwb=cwb)
        if has_bias:
            m["brow"] = brow_cores[c]
            m["clb"] = clb
        in_maps.append(m)

    meta = dict(dev_orig=dev_orig, n=n, nb=nb)
    return plan, in_maps, meta


def _unshard(out, meta):
    nll_dev = np.ascontiguousarray(np.asarray(out, dtype=np.float32).T).reshape(-1)
    res = np.zeros(meta["n"], dtype=np.float32)
    real = meta["dev_orig"] >= 0
    res[meta["dev_orig"][real]] = nll_dev[real]
    return res


def kernel(x, y, cluster_w, cluster_b, logits_w, logits_b):
    plan, in_maps, meta = _shard(x, y, cluster_w, cluster_b, logits_w, logits_b)
    nc = build_graph(plan)
    res = run_bass_kernel_spmd(nc, in_maps, list(range(N_CORES)))
    return _unshard(res.results[0]["out"], meta)


# revision 45
# speedup vs baseline: 1.5778x; 1.1079x over previous
"""Adaptive-softmax NLL loss on 8 Trainium2 NeuronCores.

Algorithm (cluster-sparse): per token only its own cluster's log-softmax
matters, so
    nll[t] = -( cl[t, c(t)] - LSE(cl[t,:]) + logit[t, y_t] - ln S[t] )
with  S[t] = sum_{j in cluster(y_t)} exp(x_t . W[:,j] + b_j).

Sharding: tokens are cluster-sorted into 128-row blocks; each cluster's
vocab range is split evenly across the 8 cores (tensor parallel over
vocab).  Every core computes partial S for all tokens over its vocab
slice (fp8 DoubleRow matmul -> ScalarE exp with free-axis accumulate),
the partials are combined with a single small AllReduce, and each core
finishes the per-token epilogue locally.  The target logit is computed
from the host-gathered columns W[:, y] as an elementwise bf16 dot on
VectorE.  The 3-column cluster head rides along as 3 extra weight
columns.  fp8 inputs are pre-scaled by powers of two on the host; the
exp's built-in scale multiplier unwinds the scaling for free.
"""

import numpy as np
import ml_dtypes
from contextlib import ExitStack

import concourse.bass as bass
import concourse.mybir as mybir
from concourse.bass_utils import run_bass_kernel_spmd

F32 = mybir.dt.float32
BF16 = mybir.dt.bfloat16
FP8 = mybir.dt.float8e4
AF = mybir.ActivationFunctionType
ALU = mybir.AluOpType
DR = mybir.MatmulPerfMode.DoubleRow
DRSW = mybir.MatmulPerfMode.DoubleRowSwInterleave
USE_SWI = False

N_CORES = 8
PART = 128
CUTOFFS = [0, 2000, 10000, 50000]
HID = 512

GROUP_COLS = 1024   # retained for the small-scale sim configs
TCAPS = [1024, 1024, 1024, 1024]   # psum tensor widths (2 banks each)
MM_F = 512          # max matmul free size (one psum bank)
SCALE_W = 2048.0    # fp8 pre-scale for weights (power of 2)
SCALE_X = 32.0      # fp8 pre-scale for activations (power of 2)

DISABLE = set()     # bisection hooks


# ---------------------------------------------------------------------------
# planning


class Plan:
    """Static schedule shared by the host sharding code and the builder."""

    def __init__(self, blocks_per_cluster, widths, has_bias, group_cols=GROUP_COLS,
                 hid=HID, mm_f=MM_F):
        assert hid % 256 == 0
        self.hg = hid // PART          # 128-row h-groups (4)
        self.ndr = hid // 256          # DoubleRow matmuls per unit (2)
        self.hid = hid
        self.has_bias = has_bias
        self.group_cols = group_cols
        self.mm_f = mm_f
        self.widths = widths                      # per-core cols per cluster
        self.bpc = blocks_per_cluster             # blocks per cluster
        self.nb = sum(blocks_per_cluster)
        self.ncl = len(widths)
        self.act_scale = 1.0 / (SCALE_W * SCALE_X)

        # per-core w column layout: [c0 | c1 | ... ] (cluster head is
        # computed on VectorE from bf16 inputs instead)
        self.w_off = []
        off = 0
        for wd in widths:
            self.w_off.append(off)
            off += wd
        self.wcols = off

        # head-split: part0 = first cluster only; part1 = all clusters
        # except the last; part2 = the big last cluster
        self.wsplit0 = sum(widths[:-1])
        self.tsplit0 = PART * sum(blocks_per_cluster[:-1])
        self.wsplit = self.w_off[-1]
        self.tsplit = PART * sum(blocks_per_cluster[:-1])

        # blocks: cluster index per block
        self.block_cluster = []
        for ci, nblk in enumerate(blocks_per_cluster):
            self.block_cluster += [ci] * nblk

        # groups: the unit of PSUM rotation.  Asymmetric psum tensors,
        # assigned round-robin (LRU); each group is one ACT exp+accum.
        if group_cols == GROUP_COLS:
            self.tcaps = list(TCAPS)
        else:                      # small-scale sim: 4 tensors of group_cols
            self.tcaps = [group_cols] * 4
        self.groups = []   # dicts: b, gi, tidx, prev_g, units[(po,wo,F)], span
        lru = list(range(len(self.tcaps)))
        last_on = [None] * len(self.tcaps)
        for b, ci in enumerate(self.block_cluster):
            V = widths[ci]
            wo0 = self.w_off[ci]
            col = 0
            gi = 0
            while col < V:
                t = lru.pop(0)
                lru.append(t)
                gsz = min(self.tcaps[t], V - col)
                units = []
                po = 0
                rem = gsz
                while rem > 0:
                    f = min(self.mm_f, rem)
                    units.append((po, wo0 + col + po, f))
                    po += f
                    rem -= f
                g = len(self.groups)
                self.groups.append(dict(b=b, gi=gi, tidx=t, prev_g=last_on[t],
                                        units=units, span=gsz))
                last_on[t] = g
                col += gsz
                gi += 1
        self.ngroups = len(self.groups)

        # fuse ACT over pairs of groups in adjacent psum quarters (the psum
        # is one contiguous tensor; consecutive tidx => contiguous columns)
        self.act_instrs = []    # dicts: b, span_off, span, slot, last_g
        self.act_of_group = [None] * self.ngroups
        caps = self.tcaps
        g = 0
        while g < self.ngroups:
            grp = self.groups[g]
            b = grp["b"]
            fuse = False
            if g + 1 < self.ngroups:
                nxt = self.groups[g + 1]
                if (nxt["b"] == b and nxt["tidx"] == grp["tidx"] + 1
                        and grp["span"] == caps[grp["tidx"]]):
                    fuse = True
            off = sum(caps[:grp["tidx"]])
            if fuse:
                span = grp["span"] + self.groups[g + 1]["span"]
                idx = len(self.act_instrs)
                self.act_of_group[g] = idx
                self.act_of_group[g + 1] = idx
                slot = len([a for a in self.act_instrs if a["b"] == b])
                self.act_instrs.append(dict(b=b, span_off=off, span=span,
                                            slot=slot, last_g=g + 1))
                g += 2
            else:
                idx = len(self.act_instrs)
                self.act_of_group[g] = idx
                slot = len([a for a in self.act_instrs if a["b"] == b])
                self.act_instrs.append(dict(b=b, span_off=off, span=grp["span"],
                                            slot=slot, last_g=g))
                g += 1
        self.n_act = len(self.act_instrs)
        self.max_gpb = max(a["slot"] for a in self.act_instrs) + 1

        # first group needing part1 (middle clusters) / part2 (last cluster)
        self.first_p1_group = None
        self.first_p2_group = None
        for g, grp in enumerate(self.groups):
            ci = self.block_cluster[grp["b"]]
            if ci not in (0, self.ncl - 1) and self.first_p1_group is None:
                self.first_p1_group = g
            if ci == self.ncl - 1 and self.first_p2_group is None:
                self.first_p2_group = g
                break


def build_graph(plan: Plan):
    nc = bass.Bass()
    HG, NB, G = plan.hg, plan.nb, plan.ngroups
    NTOK = NB * PART
    W = plan.wcols
    WS, TS = plan.wsplit, plan.tsplit

    if USE_SWI:
        x8_ext = nc.declare_dram_parameter("x8", [PART, NB, plan.ndr, 2 * PART],
                                           FP8, isOutput=False)
    else:
        x8_ext = nc.declare_dram_parameter("x8", [PART, HG, NTOK], FP8,
                                           isOutput=False)
    w8_ext = nc.declare_dram_parameter("w8", [PART, HG, W], FP8, isOutput=False)
    xe_ext = nc.declare_dram_parameter("xe", [NTOK, plan.hid], BF16, isOutput=False)
    wt_ext = nc.declare_dram_parameter("wt", [NTOK, plan.hid], BF16, isOutput=False)
    cwb_ext = nc.declare_dram_parameter("cwb", [PART, 3 * plan.hid], BF16,
                                        isOutput=False)
    oh_ext = nc.declare_dram_parameter("oh", [PART, NB, 3], F32, isOutput=False)
    bt_ext = nc.declare_dram_parameter("bt", [PART, NB], F32, isOutput=False)
    if plan.has_bias:
        brow_ext = nc.declare_dram_parameter("brow", [1, W], BF16, isOutput=False)
        clb_ext = nc.declare_dram_parameter("clb", [PART, NB, 3], F32,
                                            isOutput=False)
    out_ext = nc.declare_dram_parameter("out", [PART, NB], F32, isOutput=True)

    ar_in = nc.dram_tensor("ar_in", [PART, NB], F32)
    ar_out = nc.dram_tensor("ar_out", [PART, NB], F32, addr_space="Shared")
    dm_in = nc.dram_tensor("dm_in", [PART], F32)
    dm_out = nc.dram_tensor("dm_out", [PART], F32, addr_space="Shared")

    n_p0 = 1 + (1 if plan.tsplit0 > 0 else 0) + (1 if plan.has_bias else 0)
    n_p1 = ((1 if plan.wsplit > plan.wsplit0 else 0)
            + (1 if plan.tsplit > plan.tsplit0 else 0))
    n_misc = 2                               # oh, bt

    with ExitStack() as ctx:
        w8_sb = ctx.enter_context(nc.sbuf_tensor([PART, HG, W], FP8))
        if USE_SWI:
            x8_sb = ctx.enter_context(
                nc.sbuf_tensor([PART, NB * plan.ndr * 2 * PART], FP8))
        else:
            x8_sb = ctx.enter_context(nc.sbuf_tensor([PART, HG, NTOK], FP8))
        xe_sb = ctx.enter_context(nc.sbuf_tensor([PART, 2 * plan.hid], BF16))
        wt_sb = ctx.enter_context(nc.sbuf_tensor([PART, 2 * plan.hid], BF16))
        sacc_sb = ctx.enter_context(nc.sbuf_tensor([PART, NB, plan.max_gpb], F32))
        cl_sb = ctx.enter_context(nc.sbuf_tensor([PART, NB, 3], F32))
        ecl_sb = ctx.enter_context(nc.sbuf_tensor([PART, NB, 3], F32))
        oh_sb = ctx.enter_context(nc.sbuf_tensor([PART, NB, 3], F32))
        tmp3_sb = ctx.enter_context(nc.sbuf_tensor([PART, NB, 3], F32))
        prod_sb = ctx.enter_context(nc.sbuf_tensor([PART, 8 * plan.hid], F32))
        cwb_sb = ctx.enter_context(nc.sbuf_tensor([PART, 3 * plan.hid], BF16))
        t_sb = ctx.enter_context(nc.sbuf_tensor([PART, NB], F32))
        bt_sb = ctx.enter_context(nc.sbuf_tensor([PART, NB], F32))
        s_sb = ctx.enter_context(nc.sbuf_tensor([PART, NB], F32))
        st_sb = ctx.enter_context(nc.sbuf_tensor([PART, NB], F32))
        lns_sb = ctx.enter_context(nc.sbuf_tensor([PART, NB], F32))
        se3_sb = ctx.enter_context(nc.sbuf_tensor([PART, NB], F32))
        lse3_sb = ctx.enter_context(nc.sbuf_tensor([PART, NB], F32))
        clsel_sb = ctx.enter_context(nc.sbuf_tensor([PART, NB], F32))
        fin_sb = ctx.enter_context(nc.sbuf_tensor([PART, NB], F32))
        ones_sb = ctx.enter_context(nc.sbuf_tensor([1, PART], BF16))
        brow_sb = ctx.enter_context(nc.sbuf_tensor([1, W], BF16))
        ps = ctx.enter_context(nc.psum_tensor("ps",
                                              [PART, sum(plan.tcaps)], F32))
        pbase = [sum(plan.tcaps[:i]) for i in range(len(plan.tcaps))]
        dma_w0 = ctx.enter_context(nc.semaphore("dma_w0"))
        dma_w1 = ctx.enter_context(nc.semaphore("dma_w1"))
        dma_w2 = ctx.enter_context(nc.semaphore("dma_w2"))
        dma_misc = ctx.enter_context(nc.semaphore("dma_misc"))
        dma_ep0 = ctx.enter_context(nc.semaphore("dma_ep0"))
        dma_ep1 = ctx.enter_context(nc.semaphore("dma_ep1"))
        dma_out = ctx.enter_context(nc.semaphore("dma_out"))
        mm_sem = ctx.enter_context(nc.semaphore("mm_sem"))
        act_sem = ctx.enter_context(nc.semaphore("act_sem"))
        dma_cwb = ctx.enter_context(nc.semaphore("dma_cwb"))
        tdot_sem = ctx.enter_context(nc.semaphore("tdot_sem"))
        veini_sem = ctx.enter_context(nc.semaphore("veini_sem"))
        ve_sem = ctx.enter_context(nc.semaphore("ve_sem"))
        ve2_sem = ctx.enter_context(nc.semaphore("ve2_sem"))
        cc_sem = ctx.enter_context(nc.semaphore("cc_sem"))
        fin_sem = ctx.enter_context(nc.semaphore("fin_sem"))
        outv_sem = ctx.enter_context(nc.semaphore("outv_sem"))
        vchain_sem = ctx.enter_context(nc.semaphore("vchain_sem"))
        gp_sem = ctx.enter_context(nc.semaphore("gp_sem"))
        block = ctx.enter_context(nc.Block())

        WS0, TS0 = plan.wsplit0, plan.tsplit0

        @block.sync
        def _(sync):
            def x8_dma(sem, tok_lo, tok_hi):
                if USE_SWI:
                    blo, bhi = tok_lo // PART, tok_hi // PART
                    clo, chi = blo * plan.ndr * 2 * PART, bhi * plan.ndr * 2 * PART
                    sync.dma_start(
                        out=x8_sb[:, clo:chi],
                        in_=x8_ext[:, blo:bhi, :, :]).then_inc(sem, 16)
                else:
                    sync.dma_start(out=x8_sb[:, :, tok_lo:tok_hi],
                                   in_=x8_ext[:, :, tok_lo:tok_hi]).then_inc(sem, 16)

            # part 0: just the first cluster's slice, to start PE asap
            sync.dma_start(out=w8_sb[:, :, 0:WS0],
                           in_=w8_ext[:, :, 0:WS0]).then_inc(dma_w0, 16)
            if TS0 > 0:
                x8_dma(dma_w0, 0, TS0)
            # part 1: remaining small clusters (empty when no middle part)
            if WS > WS0:
                sync.dma_start(out=w8_sb[:, :, WS0:WS],
                               in_=w8_ext[:, :, WS0:WS]).then_inc(dma_w1, 16)
            if TS > TS0:
                x8_dma(dma_w1, TS0, TS)
            if plan.has_bias:
                sync.dma_start(out=brow_sb[:], in_=brow_ext[:]).then_inc(dma_w0, 16)
            sync.dma_start(out=cwb_sb[:], in_=cwb_ext[:]).then_inc(dma_cwb, 16)
            if plan.has_bias:
                sync.dma_start(out=tmp3_sb[:], in_=clb_ext[:]).then_inc(dma_cwb, 16)
            # part 2: the big cluster
            sync.dma_start(out=w8_sb[:, :, WS:W],
                           in_=w8_ext[:, :, WS:W]).then_inc(dma_w2, 16)
            x8_dma(dma_w2, TS, NTOK)
            # misc for the epilogue
            sync.dma_start(out=oh_sb[:], in_=oh_ext[:]).then_inc(dma_misc, 16)
            sync.dma_start(out=bt_sb[:], in_=bt_ext[:]).then_inc(dma_misc, 16)
            # epilogue tiles, double-buffered, paced by the t-dot consumer
            for e in range(NB):
                if e >= 2:
                    sync.wait_ge(tdot_sem, 4 * (e - 1))
                sem_e = dma_ep0 if e % 2 == 0 else dma_ep1
                toff = (e % 2) * plan.hid
                sync.dma_start(out=xe_sb[:, toff:toff + plan.hid],
                               in_=xe_ext[e * PART:(e + 1) * PART, :]
                               ).then_inc(sem_e, 16)
                sync.dma_start(out=wt_sb[:, toff:toff + plan.hid],
                               in_=wt_ext[e * PART:(e + 1) * PART, :]
                               ).then_inc(sem_e, 16)
            # S partials out, AllReduce result back, final output
            sync.wait_ge(ve_sem, 1)
            sync.dma_start(out=ar_in[:], in_=s_sb[:]).then_inc(dma_out, 16)
            sync.wait_ge(cc_sem, 4)
            sync.dma_start(out=st_sb[:], in_=ar_out[:]).then_inc(dma_out, 16)
            sync.wait_ge(outv_sem, 1)
            sync.dma_start(out=out_ext[:], in_=fin_sb[:]).then_inc(dma_out, 16)

        @block.gpsimd
        def _(gpsimd):
            # tiny dummy collective issued immediately: pays the cold-start
            # and entry-barrier cost concurrently with the main compute, so
            # the real AllReduce at the end runs on a warm path
            gpsimd.dma_start(out=dm_in[:],
                             in_=bt_ext[:].rearrange("p e -> (p e)")[0:PART]
                             ).then_inc(gp_sem, 16)
            gpsimd.wait_ge(gp_sem, 16)
            gpsimd.collective_compute(
                "AllReduce",
                ALU.add,
                ins=[dm_in[:]],
                outs=[dm_out[:]],
                replica_groups=[list(range(N_CORES))],
            ).then_inc(cc_sem, 1)
            gpsimd.wait_ge(mm_sem, (G * 11) // 20)
            gpsimd.collective_compute(
                "AllReduce",
                ALU.add,
                ins=[dm_in[:]],
                outs=[dm_out[:]],
                replica_groups=[list(range(N_CORES))],
            ).then_inc(cc_sem, 1)
            gpsimd.wait_ge(mm_sem, (G * 17) // 20)
            gpsimd.collective_compute(
                "AllReduce",
                ALU.add,
                ins=[dm_in[:]],
                outs=[dm_out[:]],
                replica_groups=[list(range(N_CORES))],
            ).then_inc(cc_sem, 1)
            gpsimd.wait_ge(dma_out, 16)
            gpsimd.collective_compute(
                "AllReduce",
                ALU.add,
                ins=[ar_in[:]],
                outs=[ar_out[:]],
                replica_groups=[list(range(N_CORES))],
            ).then_inc(cc_sem, 1)

        @block.tensor
        def _(tensor):
            tensor.wait_ge(dma_w0, 16 * n_p0)
            if plan.has_bias:
                tensor.wait_ge(veini_sem, 2)  # ones row ready
            for g, grp in enumerate(plan.groups):
                pb0 = pbase[grp["tidx"]]
                if g == plan.first_p1_group and n_p1 > 0:
                    tensor.wait_ge(dma_w1, 16 * n_p1)
                if g == plan.first_p2_group:
                    tensor.wait_ge(dma_w2, 32)
                if grp["prev_g"] is not None:
                    tensor.wait_ge(act_sem,
                                   plan.act_of_group[grp["prev_g"]] + 1)
                b = grp["b"]
                nunits = len(grp["units"])
                for ui, (po, wo, f) in enumerate(grp["units"]):
                    for j in range(plan.ndr):
                        if USE_SWI:
                            xoff = (b * plan.ndr + j) * 2 * PART
                            lhsT = x8_sb[:, xoff:xoff + 2 * PART]
                        else:
                            lhsT = x8_sb[:, 2 * j:2 * j + 2,
                                         b * PART:(b + 1) * PART]
                        mm = tensor.matmul(
                            ps[:, pb0 + po:pb0 + po + f],
                            lhsT=lhsT,
                            rhs=w8_sb[:, 2 * j:2 * j + 2, wo:wo + f],
                            start=(j == 0),
                            stop=(j == plan.ndr - 1 and not plan.has_bias),
                            perf_mode=DRSW if USE_SWI else DR)
                        if (j == plan.ndr - 1 and not plan.has_bias
                                and ui == nunits - 1):
                            mm.then_inc(mm_sem, 1)
                    if plan.has_bias:
                        mm = tensor.matmul(
                            ps[:, pb0 + po:pb0 + po + f],
                            lhsT=ones_sb[:],
                            rhs=brow_sb[0:1, wo:wo + f],
                            start=False, stop=True)
                        if ui == nunits - 1:
                            mm.then_inc(mm_sem, 1)

        @block.scalar
        def _(scalar):
            scalar.wait_ge(veini_sem, 1)
            for ai, a in enumerate(plan.act_instrs):
                scalar.wait_ge(mm_sem, a["last_g"] + 1)
                o, sp = a["span_off"], a["span"]
                scalar.activation(
                    ps[:, o:o + sp],
                    ps[:, o:o + sp],
                    AF.Exp,
                    scale=plan.act_scale,
                    accum_out=sacc_sb[:, a["b"], a["slot"]:a["slot"] + 1],
                ).then_inc(act_sem, 1)
            # epilogue
            if plan.has_bias:
                scalar.wait_ge(ve2_sem, 2)
            else:
                scalar.wait_ge(tdot_sem, 4 * NB)
            scalar.activation(ecl_sb[:], cl_sb[:], AF.Exp).then_inc(fin_sem, 1)
            scalar.wait_ge(ve2_sem, 3 if plan.has_bias else 1)
            scalar.activation(lse3_sb[:], se3_sb[:], AF.Ln).then_inc(fin_sem, 1)
            scalar.wait_ge(dma_out, 32)
            scalar.activation(lns_sb[:], st_sb[:], AF.Ln).then_inc(fin_sem, 1)

        @block.vector
        def _(vector):
            vector.memset(sacc_sb[:], 0.0).then_inc(veini_sem, 1)
            if plan.has_bias:
                vector.memset(ones_sb[:], 1.0).then_inc(veini_sem, 1)
            vector.wait_ge(dma_cwb, 32 if plan.has_bias else 16)
            H = plan.hid
            for b in range(NB):
                vector.wait_ge(dma_ep0 if b % 2 == 0 else dma_ep1,
                               32 * (b // 2 + 1))
                toff = (b % 2) * H
                # target-logit dot + 3 cluster-head dots, each with its own
                # scratch slot (WAW across tiles is ordered transitively via
                # the DMA pacing)
                po = (b % 2) * 4 * H
                vector.scalar_tensor_tensor(
                    out=prod_sb[:, po:po + H],
                    in0=xe_sb[:, toff:toff + H],
                    scalar=1.0,
                    in1=wt_sb[:, toff:toff + H],
                    op0=ALU.mult,
                    op1=ALU.mult,
                    accum_out=t_sb[:, b:b + 1],
                ).then_inc(tdot_sem, 1)
                for i in range(3):
                    vector.scalar_tensor_tensor(
                        out=prod_sb[:, po + (i + 1) * H:po + (i + 2) * H],
                        in0=xe_sb[:, toff:toff + H],
                        scalar=1.0,
                        in1=cwb_sb[:, i * H:(i + 1) * H],
                        op0=ALU.mult,
                        op1=ALU.mult,
                        accum_out=cl_sb[:, b, i:i + 1],
                    ).then_inc(tdot_sem, 1)
            # ---- tail (serialized through vchain_sem for the race detector)
            vc = 0
            if plan.has_bias:
                # cl += cluster_b (clb staged in tmp3_sb)
                vector.wait_ge(tdot_sem, 4 * NB)
                vector.wait_ge(dma_cwb, 32)
                vector.tensor_tensor(cl_sb[:], cl_sb[:], tmp3_sb[:],
                                     ALU.add).then_inc(ve2_sem, 2)
            vector.wait_ge(act_sem, plan.n_act)
            vector.tensor_reduce(s_sb[:], sacc_sb[:], mybir.AxisListType.X,
                                 ALU.add).then_inc(ve_sem, 1)
            # cluster-head select (overlaps the AllReduce)
            vector.wait_ge(dma_misc, 16 * n_misc)
            if plan.has_bias:
                vector.wait_ge(ve2_sem, 2)
            else:
                vector.wait_ge(tdot_sem, 4 * NB)
            vector.tensor_tensor(tmp3_sb[:], cl_sb[:], oh_sb[:],
                                 ALU.mult).then_inc(vchain_sem, 1)
            vc += 1
            vector.wait_ge(vchain_sem, vc)
            vector.tensor_reduce(clsel_sb[:], tmp3_sb[:], mybir.AxisListType.X,
                                 ALU.add).then_inc(vchain_sem, 1)
            vc += 1
            vector.wait_ge(fin_sem, 1)
            vector.tensor_reduce(se3_sb[:], ecl_sb[:], mybir.AxisListType.X,
                                 ALU.add).then_inc(ve2_sem, 1)
            # pre-AR: w = lse3 - clsel - t - bt  (staged in lse3_sb)
            vector.wait_ge(fin_sem, 2)
            vector.scalar_tensor_tensor(out=lse3_sb[:], in0=lse3_sb[:], scalar=1.0,
                                        in1=clsel_sb[:], op0=ALU.mult,
                                        op1=ALU.subtract).then_inc(vchain_sem, 1)
            vc += 1
            vector.wait_ge(vchain_sem, vc)
            vector.scalar_tensor_tensor(out=lse3_sb[:], in0=lse3_sb[:], scalar=1.0,
                                        in1=t_sb[:], op0=ALU.mult,
                                        op1=ALU.subtract).then_inc(vchain_sem, 1)
            vc += 1
            vector.wait_ge(vchain_sem, vc)
            vector.scalar_tensor_tensor(out=lse3_sb[:], in0=lse3_sb[:], scalar=1.0,
                                        in1=bt_sb[:], op0=ALU.mult,
                                        op1=ALU.subtract).then_inc(vchain_sem, 1)
            vc += 1
            # post-AR: nll = lnS + w
            vector.wait_ge(fin_sem, 3)
            vector.wait_ge(vchain_sem, vc)
            vector.scalar_tensor_tensor(out=fin_sb[:], in0=lns_sb[:], scalar=1.0,
                                        in1=lse3_sb[:], op0=ALU.mult,
                                        op1=ALU.add).then_inc(outv_sem, 1)

    return nc


# ---------------------------------------------------------------------------
# host side


def _fp8(a, scale):
    return np.clip(np.asarray(a, np.float32) * scale, -240.0, 240.0).astype(
        ml_dtypes.float8_e4m3)


def _shard(x, y, cluster_w, cluster_b, logits_w, logits_b, cuts=CUTOFFS,
           group_cols=GROUP_COLS, mm_f=MM_F):
    x = np.asarray(x)
    y = np.asarray(y)
    cluster_w = np.asarray(cluster_w, dtype=np.float32)
    cluster_b = np.asarray(cluster_b, dtype=np.float32)
    logits_w = np.asarray(logits_w, dtype=np.float32)
    logits_b = np.asarray(logits_b, dtype=np.float32)

    xf = np.ascontiguousarray(x[:, :-1]).reshape(-1, x.shape[-1]).astype(np.float32)
    yf = y.reshape(-1).astype(np.int64)
    n = xf.shape[0]
    hid = xf.shape[1]
    ncl = len(cuts) - 1
    hg = hid // PART

    cid = np.zeros(n, dtype=np.int64)
    for i in range(1, ncl):
        cid += yf >= cuts[i]

    order = np.argsort(cid, kind="stable")
    counts = np.bincount(cid, minlength=ncl)
    bpc = [int(-(-c // PART)) for c in counts]
    nb = sum(bpc)
    ntok = nb * PART

    dev_orig = np.full(ntok, -1, dtype=np.int64)
    y_dev = np.empty(ntok, dtype=np.int64)
    cid_dev = np.empty(ntok, dtype=np.int64)
    pos = 0
    spos = 0
    for ci in range(ncl):
        cnt = int(counts[ci])
        seg = order[spos:spos + cnt]
        dev_orig[pos:pos + cnt] = seg
        y_dev[pos:pos + cnt] = yf[seg]
        y_dev[pos + cnt:pos + bpc[ci] * PART] = cuts[ci]
        cid_dev[pos:pos + bpc[ci] * PART] = ci
        pos += bpc[ci] * PART
        spos += cnt

    xf_dev = np.zeros((ntok, hid), dtype=np.float32)
    real = dev_orig >= 0
    xf_dev[real] = xf[dev_orig[real]]

    bf = ml_dtypes.bfloat16
    # fp8 DoubleRow layout: [p, g, tok] with contraction k = g*128 + p
    x8g = _fp8(xf_dev.T, SCALE_X).reshape(hg, PART, ntok)
    if USE_SWI:
        # DoubleRowSwInterleave stationary layout: per (block b, pair j):
        # sw[p, 2k+i] = x[(2j+i)*128+p, b*128 + (127-k)]
        ndr = hg // 2
        a = x8g.reshape(ndr, 2, PART, nb, PART)        # [j, i, p, b, tok]
        a = a[:, :, :, :, ::-1]                        # reverse tokens
        # -> [p, b, j, tok, i]
        a = a.transpose(2, 3, 0, 4, 1)
        x8 = np.ascontiguousarray(a.reshape(PART, nb, ndr, 2 * PART))
    else:
        x8 = np.ascontiguousarray(x8g.transpose(1, 0, 2))
    xe = np.ascontiguousarray(xf_dev).astype(bf)             # [ntok, H]
    wt = np.ascontiguousarray(logits_w.T[y_dev]).astype(bf)  # [ntok, H]

    bt = logits_b[0, y_dev].astype(np.float32).reshape(nb, PART).T.copy()
    oh = np.zeros((ntok, 3), dtype=np.float32)
    oh[np.arange(ntok), cid_dev] = 1.0
    oh = np.ascontiguousarray(oh.reshape(nb, PART, 3).transpose(1, 0, 2))

    has_bias = bool(logits_b.any() or cluster_b.any())
    widths = []
    for ci in range(ncl):
        v = cuts[ci + 1] - cuts[ci]
        assert v % N_CORES == 0
        widths.append(v // N_CORES)

    cwb = np.ascontiguousarray(np.broadcast_to(
        cluster_w.T.reshape(1, 3 * hid), (PART, 3 * hid))).astype(bf)
    clb = np.ascontiguousarray(np.broadcast_to(
        cluster_b.reshape(1, 1, 3), (PART, nb, 3))).astype(np.float32)

    w_cores = []
    brow_cores = []
    bscale = SCALE_W * SCALE_X
    for c in range(N_CORES):
        parts = []
        bparts = []
        for ci in range(ncl):
            lo = cuts[ci] + c * widths[ci]
            parts.append(logits_w[:, lo:lo + widths[ci]])
            bparts.append(logits_b[:, lo:lo + widths[ci]] * bscale)
        wc = np.concatenate(parts, 1)                       # [hid, W]
        w8 = np.ascontiguousarray(
            _fp8(wc, SCALE_W).reshape(hg, PART, -1).transpose(1, 0, 2))
        w_cores.append(w8)
        brow_cores.append(np.ascontiguousarray(np.concatenate(bparts, 1)).astype(bf))

    plan = Plan(bpc, widths, has_bias, group_cols=group_cols, hid=hid, mm_f=mm_f)

    in_maps = []
    for c in range(N_CORES):
        m = dict(x8=x8, w8=w_cores[c], xe=xe, wt=wt, oh=oh, bt=bt, cwb=cwb)
        if has_bias:
            m["brow"] = brow_cores[c]
            m["clb"] = clb
        in_maps.append(m)

    meta = dict(dev_orig=dev_orig, n=n, nb=nb)
    return plan, in_maps, meta


def _unshard(out, meta):
    nll_dev = np.ascontiguousarray(np.asarray(out, dtype=np.float32).T).reshape(-1)
    res = np.zeros(meta["n"], dtype=np.float32)
    real = meta["dev_orig"] >= 0
    res[meta["dev_orig"][real]] = nll_dev[real]
    return res


def kernel(x, y, cluster_w, cluster_b, logits_w, logits_b):
    plan, in_maps, meta = _shard(x, y, cluster_w, cluster_b, logits_w, logits_b)
    nc = build_graph(plan)
    res = run_bass_kernel_spmd(nc, in_maps, list(range(N_CORES)))
    return _unshard(res.results[0]["out"], meta)
